# revision 5
# baseline (speedup 1.0000x reference)
"""Trainium2 Bass kernel for DCTLAVISBlip dc_transform (DCT -> truncate -> IDCT).

Strategy (v3: v2 symmetry-folded matmuls + DMA/tail restructure)
----------------------------------------------------------------
Math identical to v2 (see kernel_v2_baseline.py): fold the input on the
host, run Wu = [Me; Pe'] and Wv = [Mo; Po'] ([575, 288] each) against
u/v in fp16, ship y and raw a/b state halves as f16, combine on host.

v3 changes (from the v2 trace: PE busy 92.4us of a 130us kernel, with a
~20us output-DMA tail and 10.5us of input-wait stalls at the head):
  1. Output staging batches 4 batches per tile: stage [mmt, 4C] per
     (q, t, m-tile), ONE dma_start per (stage, dest) -> 40 output DMA
     calls instead of 160.  DIRECT2D issue on the sequencers was
     0.6-3us per call; this kills most of the tail.
  2. Device DRAM output layout [2, L, 4, C] (quad-major) so each
     partition row ships 4 batches x 2KB = 8KB contiguous DRAM lines
     (4x fewer, 4x fatter descriptors).  Host transposes for free.
  3. Remainder (K=288 tail, 32 rows) matmuls issue FIRST in each wave:
     they only need the small rem input tiles, so the PE starts ~4us
     earlier, and each batch's PSUM accumulation completes right after
     its 2nd full matmul, spreading drains across the wave.
  4. Warmup starts immediately (memset on vector, 18 matmuls) so the
     HAM clock-gate window (~3.4us) is warm when real inputs land.
  5. Optional K-probe matmuls at the very end (PROBES flag) to measure
     matmul cost vs K for the next iteration's design.
"""

import numpy as np

B, T, C = 64, 576, 1024
H = T // 2                   # 288, folded K
NCORES = 8
BPC = B // NCORES            # batches per core
Q = 0.8
PROBES = True

_CACHED = {}


def _dct_mat(N):
    n = np.arange(N)
    Mm = np.cos(np.pi * (2 * n[None, :] + 1) * n[:, None] / (2 * N))
    s = np.full(N, np.sqrt(2.0 / N))
    s[0] = np.sqrt(1.0 / N)
    return s[:, None] * Mm          # float64


def _build_weights(L):
    """Wu [H+ns1, 288] = [Me; pad; Pe'], Wv [H+ns2, 288] = [Mo; pad; Po'].
    The y block is zero-padded up to H=288 rows so the state block starts at
    a 32-aligned PSUM partition in every m-tile."""
    M64 = _dct_mat(T)
    Mi = _dct_mat(L)
    ke = np.arange(0, L, 2)
    ko = np.arange(1, L, 2)
    Pe = np.einsum('kj,kt->jt', Mi[ke, :], M64[ke, :])
    Po = np.einsum('kj,kt->jt', Mi[ko, :], M64[ko, :])
    ns1 = (L + 1) // 2
    ns2 = L // 2
    pe_u = np.zeros((H - len(ke), H))
    pe_v = np.zeros((H - len(ko), H))
    Wu = np.concatenate([M64[ke][:, :H], pe_u, Pe[:ns1, :H]], axis=0)
    Wv = np.concatenate([M64[ko][:, :H], pe_v, Po[:ns2, :H]], axis=0)
    return Wu, Wv


def _build_nc(L):
    """Bass program for truncation length L (574 for the seed-0 input).

    Inputs host-packed as in v2:
      xu/xv  [2, 2, 128, 4C] f16: (q, ki, p, (b c))
      xur/xvr [2, 128, C]: K-remainder rows of 4 batches packed on partitions
      wub/wvb [128, 2M]: cols (ki m); wur/wvr [128, M]: rem rows 4x-replic.
    Outputs (v3): yy/ss [2, L, 4, C] f16 -- quad-major so one dma_start per
    (q, t, m-tile, dest) ships 4 batches with 8KB-contiguous DRAM lines.
    """
    import concourse.bacc as bacc
    import concourse.mybir as mybir
    import concourse.tile as tile

    f16 = mybir.dt.float16
    f32 = mybir.dt.float32

    ns1 = (L + 1) // 2
    ns2 = L // 2
    MU = H + ns1
    MV = H + ns2
    MW = {"u": MU, "v": MV}
    YB = {"u": ns1, "v": ns2}         # y rows per transform
    NT = [(0, 512), (512, 512)]
    MM = max(MU, MV)
    MT = [(m0, min(128, MM - m0)) for m0 in range(0, MM, 128)]

    nc = bacc.Bacc("TRN2", target_bir_lowering=False, debug=False,
                   num_devices=NCORES)
    xu = nc.dram_tensor("xu", [2, 2, 128, 4 * C], f16, kind="ExternalInput")
    xv = nc.dram_tensor("xv", [2, 2, 128, 4 * C], f16, kind="ExternalInput")
    xur = nc.dram_tensor("xur", [2, 128, C], f16, kind="ExternalInput")
    xvr = nc.dram_tensor("xvr", [2, 128, C], f16, kind="ExternalInput")
    wub = nc.dram_tensor("wub", [128, 2 * MU], f16, kind="ExternalInput")
    wvb = nc.dram_tensor("wvb", [128, 2 * MV], f16, kind="ExternalInput")
    wur = nc.dram_tensor("wur", [128, MU], f16, kind="ExternalInput")
    wvr = nc.dram_tensor("wvr", [128, MV], f16, kind="ExternalInput")
    yy = nc.dram_tensor("yy", [2, L, 4, C], f16, kind="ExternalOutput")
    ss = nc.dram_tensor("ss", [2, L, 4, C], f16, kind="ExternalOutput")
    XD = {"u": (xu, xur, wub, wur), "v": (xv, xvr, wvb, wvr)}

    with tile.TileContext(nc) as tc:
        with (
            tc.tile_pool(name="wpool", bufs=1) as wpool,
            tc.tile_pool(name="xpool", bufs=1) as xpool,
            tc.tile_pool(name="opool", bufs=3) as opool,
            tc.tile_pool(name="ps", bufs=8, space="PSUM") as ps,
        ):
            # --- warmup immediately: memset on vector (idle at start) ---
            wz = wpool.tile([128, 128], f16, tag="wz", name="wz")
            nc.vector.memset(wz[:], 0.0)
            pwarm = ps.tile([128, 512], f32, tag="pt", name="pt")
            for _ in range(18):
                nc.tensor.matmul(pwarm[:, 0:128], wz[:], wz[:],
                                 start=True, stop=True)

            # --- input kicks, first-use order, spread across engines ---
            xt, rt, wt, wr = {}, {}, {}, {}

            def load_w(t, eng):
                _, _, wd, wrd = XD[t]
                w_ = wpool.tile([128, 2 * MW[t]], f16, tag=f"w{t}",
                                name=f"w{t}")
                eng.dma_start(w_[:], wd[:, :])
                wt[t] = w_
                w_ = wpool.tile([128, MW[t]], f16, tag=f"w{t}r",
                                name=f"w{t}r")
                eng.dma_start(w_[:], wrd[:, :])
                wr[t] = w_

            load_w("u", nc.scalar)
            load_w("v", nc.scalar)
            # remainder tiles first (tiny): rem matmuls lead each wave
            for q in range(2):
                for t in ("u", "v"):
                    _, rd, _, _ = XD[t]
                    r_ = xpool.tile([128, C], f16, tag=f"x{t}r{q}",
                                    name=f"x{t}r{q}")
                    nc.gpsimd.dma_start(r_[:], rd[q, :, :])
                    rt[(t, q)] = r_
            for q in range(2):
                for t in ("u", "v"):
                    xd, _, _, _ = XD[t]
                    for ki in range(2):
                        x_ = xpool.tile([128, 4 * C], f16,
                                        tag=f"x{t}{q}{ki}",
                                        name=f"x{t}{q}{ki}")
                        nc.sync.dma_start(x_[:, 0:2 * C],
                                          xd[q, ki, :, 0:2 * C])
                        xt[(t, q, ki)] = x_
                    for ki in range(2):
                        nc.sync.dma_start(xt[(t, q, ki)][:, 2 * C:4 * C],
                                          xd[q, ki, :, 2 * C:4 * C])

            def vcopy(dst, src):
                nc.vector.tensor_copy(dst, src)

            def scopy(dst, src):
                nc.scalar.copy(dst, src)

            oengs = [nc.sync, nc.gpsimd, nc.scalar]
            ok_i = 0     # output call counter (engine rotation)

            # --- compute waves: (q, t, m), 2 n-halves x 4 batches ---
            for q in range(2):
                for t in ("u", "v"):
                    mw = MW[t]
                    ybt = YB[t]
                    ykoff = 0 if t == "u" else ns1
                    for mi, (m0, mm) in enumerate(MT):
                        mmt = min(mm, mw - m0)
                        if mmt <= 0:
                            continue
                        yr = max(0, min(mmt, ybt - m0))   # y rows here
                        su = max(0, H - m0)               # state-local start
                        sr = max(0, mmt - su)             # state rows here
                        stage = opool.tile([128, 4 * C], f16,
                                           tag="o", name="o")
                        for ni, (n0, nn) in enumerate(NT):
                            # ramp: first m-tile of the run goes in 2-bank
                            # halves so compute starts on half the inputs
                            groups = ([(0, 1), (2, 3)]
                                      if (q == 0 and t == "u" and mi == 0)
                                      else [(0, 1, 2, 3)])
                            pts = {}
                            for grp in groups:
                                for bi in grp:
                                    pts[bi] = ps.tile([128, 512], f32,
                                                      tag="pt", name="pt")
                                # remainder first: small inputs, PE starts
                                # early; batch accumulation then completes
                                # right after its 2nd full matmul
                                for bi in grp:
                                    nc.tensor.matmul(
                                        pts[bi][0:mmt, :],
                                        wr[t][32 * bi:32 * bi + 32,
                                              m0:m0 + mmt],
                                        rt[(t, q)][32 * bi:32 * bi + 32,
                                                   n0:n0 + nn],
                                        start=True, stop=False,
                                        tile_position=(32 * bi, 0))
                                for ki in range(2):
                                    wsl = wt[t][:, ki * mw + m0:
                                                ki * mw + m0 + mmt]
                                    for bi in grp:
                                        nc.tensor.matmul(
                                            pts[bi][0:mmt, :],
                                            wsl,
                                            xt[(t, q, ki)][:, bi * C + n0:
                                                           bi * C + n0 + nn],
                                            start=False, stop=(ki == 1))
                            for bi in range(4):
                                cp = vcopy if bi % 2 == 0 else scopy
                                cp(stage[0:mmt, bi * C + n0:bi * C + n0 + nn],
                                   pts[bi][0:mmt, :])
                        # one dma_start per dest ships all 4 batches
                        if yr > 0:
                            d = yy[q, ykoff + m0:ykoff + m0 + yr, :, :]
                            oengs[ok_i % 3].dma_start(d, stage[0:yr, :])
                            ok_i += 1
                        if sr > 0:
                            j0 = max(0, m0 - H)
                            if t == "u":
                                d = ss[q, j0:j0 + sr, :, :]
                            else:
                                d = ss[q, ns1 + j0:ns1 + j0 + sr, :, :]
                            oengs[ok_i % 3].dma_start(d, stage[su:su + sr, :])
                            ok_i += 1

            if PROBES:
                # cost-curve probes: K in {96,72,64,48,33,128-solo},
                # M=127 keeps slice names distinct from main matmuls
                pdum = xpool.tile([128, 512], f16, tag="pdum", name="pdum")
                nc.gpsimd.memset(pdum[:], 0.0)
                for kk in (128, 96, 72, 64, 48, 33):
                    pp = ps.tile([128, 512], f32, tag="pt", name="pt")
                    nc.tensor.matmul(pp[0:127, :], wz[0:kk, 0:127],
                                     pdum[0:kk, :], start=True, stop=True)
    nc.finalize()
    return nc


def _get_nc(L):
    key = ("nc3", L)
    if key not in _CACHED:
        _CACHED[key] = _build_nc(L)
    return _CACHED[key]


def _ensure_trace_hook_safe():
    """If BASS_TRACE is set in the environment, run_bass_kernel_spmd imports
    antenv.axon_hooks, which may not exist. Install a working ctypes-based
    shim when possible, else disable tracing so the run cannot crash."""
    import os
    import sys
    import types

    if not os.environ.get("BASS_TRACE"):
        return
    try:
        import antenv.axon_hooks  # noqa: F401
        return
    except ImportError:
        pass
    try:
        from trn_agent_boot.trn_boot import _ntff_profile_via_ctypes
        hooks = types.ModuleType("antenv.axon_hooks")
        hook = _ntff_profile_via_ctypes("/opt/axon/libaxon_pjrt.so")
        hooks.get_axon_ntff_profile_hook = lambda: hook
        hooks.set_axon_ntff_profile_hook = lambda h: None
        sys.modules["antenv.axon_hooks"] = hooks
    except Exception:
        os.environ["BASS_NEVER_TRACE"] = "1"


def kernel(x: np.ndarray):
    from concourse.bass_utils import run_bass_kernel_spmd

    _ensure_trace_hook_safe()
    x = np.ascontiguousarray(np.asarray(x, dtype=np.float32))
    assert x.shape == (B, T, C)

    # ---- host: data-dependent truncation length L (tiny, exact math) ----
    M64 = _dct_mat(T)
    xbar = x.astype(np.float64).mean(axis=(0, 2))
    vq = np.abs(M64 @ xbar)
    thr = np.abs(np.quantile(vq, Q))
    idxs = np.where(vq > thr)[0]
    last_index = int(idxs[-1]) if idxs.size > 0 else -1
    L = last_index if last_index >= 0 else T - 1

    ns1 = (L + 1) // 2
    Wu, Wv = _build_weights(L)              # [H+ns1, 288], [H+ns2, 288]
    wu16 = np.ascontiguousarray(Wu.T).astype(np.float16)   # [288, H+ns1]
    wv16 = np.ascontiguousarray(Wv.T).astype(np.float16)

    # ---- host: fold input ----
    xf = x[:, :H, :]
    xr = x[:, T - 1:H - 1:-1, :]
    u16 = (xf + xr).astype(np.float16)
    v16 = (xf - xr).astype(np.float16)

    nc = _get_nc(L)

    def pack_x(z16):
        # [BPC,288,C] -> [2,2,128,4C] (q, ki, p, (b c)) + rem [2,128,C]
        full = z16[:, :256].reshape(2, 4, 2, 128, C)
        full = np.ascontiguousarray(full.transpose(0, 2, 3, 1, 4)
                                    ).reshape(2, 2, 128, 4 * C)
        remn = np.ascontiguousarray(z16[:, 256:288]).reshape(2, 128, C)
        return full, remn

    def pack_w(w16):
        # [288, M] -> [128, 2M] cols (ki m) + rem rows replicated [128, M]
        full = np.ascontiguousarray(w16[:256].reshape(2, 128, w16.shape[1])
                                    .transpose(1, 0, 2)
                                    ).reshape(128, 2 * w16.shape[1])
        remn = np.ascontiguousarray(np.tile(w16[256:288], (4, 1)))
        return full, remn

    wub_h, wur_h = pack_w(wu16)
    wvb_h, wvr_h = pack_w(wv16)
    in_maps = []
    for i in range(NCORES):
        xu_h, xur_h = pack_x(u16[i * BPC:(i + 1) * BPC])
        xv_h, xvr_h = pack_x(v16[i * BPC:(i + 1) * BPC])
        in_maps.append({"xu": xu_h, "xv": xv_h, "xur": xur_h, "xvr": xvr_h,
                        "wub": wub_h, "wvb": wvb_h,
                        "wur": wur_h, "wvr": wvr_h})
    res = run_bass_kernel_spmd(nc, in_maps, list(range(NCORES)))
    _CACHED["last_exec_time_ns"] = res.exec_time_ns

    # device layout [2, L, 4, C] (quad-major) -> [BPC, L, C] per core
    yy = np.concatenate(
        [res.results[i]["yy"].transpose(0, 2, 1, 3).reshape(BPC, L, C)
         for i in range(NCORES)], axis=0)
    ss = np.concatenate(
        [res.results[i]["ss"].transpose(0, 2, 1, 3).reshape(BPC, L, C)
         for i in range(NCORES)], axis=0)

    x_dct_trunc = np.empty((B, L, C), dtype=np.float32)
    x_dct_trunc[:, 0::2, :] = yy[:, :ns1, :].astype(np.float32)
    x_dct_trunc[:, 1::2, :] = yy[:, ns1:, :].astype(np.float32)
    a32 = ss[:, :ns1, :].astype(np.float32)
    b32 = ss[:, ns1:, :].astype(np.float32)
    ns2 = L // 2
    state = np.empty((B, L, C), dtype=np.float16)
    state[:, :ns2, :] = (a32[:, :ns2] + b32).astype(np.float16)
    if ns1 > ns2:
        state[:, ns2:ns1, :] = ss[:, ns2:ns1, :]   # lone middle row, L odd
    state[:, ns1:, :] = (a32[:, :ns2] - b32).astype(np.float16)[:, ::-1, :]
    return state, x_dct_trunc


# revision 12
# speedup vs baseline: 1.1598x; 1.1598x over previous
"""Trainium2 Bass kernel for DCTLAVISBlip dc_transform (DCT -> truncate -> IDCT).

Strategy (v3: v2 symmetry-folded matmuls + DMA/tail restructure)
----------------------------------------------------------------
Math identical to v2 (see kernel_v2_baseline.py): fold the input on the
host, run Wu = [Me; Pe'] and Wv = [Mo; Po'] ([575, 288] each) against
u/v in fp16, ship y and raw a/b state halves as f16, combine on host.

v3 changes (from the v2 trace: PE busy 92.4us of a 130us kernel, with a
~20us output-DMA tail and 10.5us of input-wait stalls at the head):
  1. Output staging batches 4 batches per tile: stage [mmt, 4C] per
     (q, t, m-tile), ONE dma_start per (stage, dest) -> 40 output DMA
     calls instead of 160.  DIRECT2D issue on the sequencers was
     0.6-3us per call; this kills most of the tail.
  2. Device DRAM output layout [2, L, 4, C] (quad-major) so each
     partition row ships 4 batches x 2KB = 8KB contiguous DRAM lines
     (4x fewer, 4x fatter descriptors).  Host transposes for free.
  3. Remainder (K=288 tail, 32 rows) matmuls issue FIRST in each wave:
     they only need the small rem input tiles, so the PE starts ~4us
     earlier, and each batch's PSUM accumulation completes right after
     its 2nd full matmul, spreading drains across the wave.
  4. Warmup starts immediately (memset on vector, 18 matmuls) so the
     HAM clock-gate window (~3.4us) is warm when real inputs land.
  5. Optional K-probe matmuls at the very end (PROBES flag) to measure
     matmul cost vs K for the next iteration's design.
"""

import numpy as np

B, T, C = 64, 576, 1024
H = T // 2                   # 288, folded K
NCORES = 8
BPC = B // NCORES            # batches per core
Q = 0.8
PROBES = True

_CACHED = {}


def _dct_mat(N):
    n = np.arange(N)
    Mm = np.cos(np.pi * (2 * n[None, :] + 1) * n[:, None] / (2 * N))
    s = np.full(N, np.sqrt(2.0 / N))
    s[0] = np.sqrt(1.0 / N)
    return s[:, None] * Mm          # float64


def _build_weights(L):
    """Wu [H+ns1, 288] = [Me; pad; Pe'], Wv [H+ns2, 288] = [Mo; pad; Po'].
    The y block is zero-padded up to H=288 rows so the state block starts at
    a 32-aligned PSUM partition in every m-tile."""
    M64 = _dct_mat(T)
    Mi = _dct_mat(L)
    ke = np.arange(0, L, 2)
    ko = np.arange(1, L, 2)
    Pe = np.einsum('kj,kt->jt', Mi[ke, :], M64[ke, :])
    Po = np.einsum('kj,kt->jt', Mi[ko, :], M64[ko, :])
    ns1 = (L + 1) // 2
    ns2 = L // 2
    pe_u = np.zeros((H - len(ke), H))
    pe_v = np.zeros((H - len(ko), H))
    Wu = np.concatenate([M64[ke][:, :H], pe_u, Pe[:ns1, :H]], axis=0)
    Wv = np.concatenate([M64[ko][:, :H], pe_v, Po[:ns2, :H]], axis=0)
    return Wu, Wv


def _build_nc(L):
    """Bass program for truncation length L (574 for the seed-0 input).

    Inputs host-packed as in v2:
      xu/xv  [2, 2, 128, 4C] f16: (q, ki, p, (b c))
      xur/xvr [2, 128, C]: K-remainder rows of 4 batches packed on partitions
      wub/wvb [128, 2M]: cols (ki m); wur/wvr [128, M]: rem rows 4x-replic.
    Outputs (v3): yy/ss [2, L, 4, C] f16 -- quad-major so one dma_start per
    (q, t, m-tile, dest) ships 4 batches with 8KB-contiguous DRAM lines.
    """
    import concourse.bacc as bacc
    import concourse.mybir as mybir
    import concourse.tile as tile

    f16 = mybir.dt.float16
    f32 = mybir.dt.float32

    ns1 = (L + 1) // 2
    ns2 = L // 2
    MU = H + ns1
    MV = H + ns2
    MW = {"u": MU, "v": MV}
    YB = {"u": ns1, "v": ns2}         # y rows per transform
    NT = [(0, 512), (512, 512)]
    MM = max(MU, MV)
    MT = [(m0, min(128, MM - m0)) for m0 in range(0, MM, 128)]

    nc = bacc.Bacc("TRN2", target_bir_lowering=False, debug=False,
                   num_devices=NCORES)
    xu = nc.dram_tensor("xu", [2, 2, 128, 4 * C], f16, kind="ExternalInput")
    xv = nc.dram_tensor("xv", [2, 2, 128, 4 * C], f16, kind="ExternalInput")
    xur = nc.dram_tensor("xur", [2, 128, C], f16, kind="ExternalInput")
    xvr = nc.dram_tensor("xvr", [2, 128, C], f16, kind="ExternalInput")
    wub = nc.dram_tensor("wub", [128, 2 * MU], f16, kind="ExternalInput")
    wvb = nc.dram_tensor("wvb", [128, 2 * MV], f16, kind="ExternalInput")
    wur = nc.dram_tensor("wur", [128, MU], f16, kind="ExternalInput")
    wvr = nc.dram_tensor("wvr", [128, MV], f16, kind="ExternalInput")
    yy = nc.dram_tensor("yy", [2, L, 4, C], f16, kind="ExternalOutput")
    ss = nc.dram_tensor("ss", [2, L, 4, C], f16, kind="ExternalOutput")
    XD = {"u": (xu, xur, wub, wur), "v": (xv, xvr, wvb, wvr)}

    with tile.TileContext(nc) as tc:
        with (
            tc.tile_pool(name="wpool", bufs=1) as wpool,
            tc.tile_pool(name="xpool", bufs=1) as xpool,
            tc.tile_pool(name="opool", bufs=6) as opool,
            tc.tile_pool(name="ps", bufs=8, space="PSUM") as ps,
        ):
            # --- warmup immediately: memset on vector (idle at start) ---
            wz = wpool.tile([128, 128], f16, tag="wz", name="wz")
            nc.vector.memset(wz[:], 0.0)
            pwarm = ps.tile([128, 512], f32, tag="pt", name="pt")
            for _ in range(18):
                nc.tensor.matmul(pwarm[:, 0:128], wz[:], wz[:],
                                 start=True, stop=True)

            # --- input kicks, first-use order, spread across engines ---
            xt, rt, wt, wr = {}, {}, {}, {}

            def load_w(t, eng):
                _, _, wd, wrd = XD[t]
                w_ = wpool.tile([128, 2 * MW[t]], f16, tag=f"w{t}",
                                name=f"w{t}")
                eng.dma_start(w_[:], wd[:, :])
                wt[t] = w_
                w_ = wpool.tile([128, MW[t]], f16, tag=f"w{t}r",
                                name=f"w{t}r")
                eng.dma_start(w_[:], wrd[:, :])
                wr[t] = w_

            load_w("u", nc.scalar)
            load_w("v", nc.scalar)
            for q in range(2):
                for t in ("u", "v"):
                    xd, rd, _, _ = XD[t]
                    for ki in range(2):
                        x_ = xpool.tile([128, 4 * C], f16,
                                        tag=f"x{t}{q}{ki}",
                                        name=f"x{t}{q}{ki}")
                        nc.sync.dma_start(x_[:, 0:2 * C],
                                          xd[q, ki, :, 0:2 * C])
                        xt[(t, q, ki)] = x_
                    r_ = xpool.tile([128, C], f16, tag=f"x{t}r{q}",
                                    name=f"x{t}r{q}")
                    nc.gpsimd.dma_start(r_[:], rd[q, :, :])
                    rt[(t, q)] = r_
                    for ki in range(2):
                        nc.sync.dma_start(xt[(t, q, ki)][:, 2 * C:4 * C],
                                          xd[q, ki, :, 2 * C:4 * C])

            def vcopy(dst, src):
                nc.vector.tensor_copy(dst, src)

            def scopy(dst, src):
                nc.scalar.copy(dst, src)

            oengs = [nc.sync, nc.gpsimd]
            ok_i = 0     # output call counter (engine rotation)

            # --- compute waves: (q, t, m), 2 n-halves x 4 batches ---
            for q in range(2):
                for t in ("u", "v"):
                    mw = MW[t]
                    ybt = YB[t]
                    ykoff = 0 if t == "u" else ns1
                    for mi, (m0, mm) in enumerate(MT):
                        mmt = min(mm, mw - m0)
                        if mmt <= 0:
                            continue
                        yr = max(0, min(mmt, ybt - m0))   # y rows here
                        su = max(0, H - m0)               # state-local start
                        sr = max(0, mmt - su)             # state rows here
                        stage = opool.tile([128, 4 * C], f16,
                                           tag="o", name="o")
                        for ni, (n0, nn) in enumerate(NT):
                            # ramp: first m-tile of the run goes in 2-bank
                            # halves so compute starts on half the inputs
                            groups = ([(0, 1), (2, 3)]
                                      if (q == 0 and t == "u" and mi == 0)
                                      else [(0, 1, 2, 3)])
                            pts = {}
                            for grp in groups:
                                for bi in grp:
                                    pts[bi] = ps.tile([128, 512], f32,
                                                      tag="pt", name="pt")
                                for ki in range(2):
                                    wsl = wt[t][:, ki * mw + m0:
                                                ki * mw + m0 + mmt]
                                    for bi in grp:
                                        nc.tensor.matmul(
                                            pts[bi][0:mmt, :],
                                            wsl,
                                            xt[(t, q, ki)][:, bi * C + n0:
                                                           bi * C + n0 + nn],
                                            start=(ki == 0), stop=False)
                                for bi in grp:
                                    nc.tensor.matmul(
                                        pts[bi][0:mmt, :],
                                        wr[t][32 * bi:32 * bi + 32,
                                              m0:m0 + mmt],
                                        rt[(t, q)][32 * bi:32 * bi + 32,
                                                   n0:n0 + nn],
                                        start=False, stop=True,
                                        tile_position=(32 * bi, 0))
                            for bi in range(4):
                                cp = vcopy if bi % 2 == 0 else scopy
                                cp(stage[0:mmt, bi * C + n0:bi * C + n0 + nn],
                                   pts[bi][0:mmt, :])
                        # one dma_start per dest ships all 4 batches
                        if yr > 0:
                            d = yy[q, ykoff + m0:ykoff + m0 + yr, :, :]
                            oengs[ok_i % 2].dma_start(d, stage[0:yr, :])
                            ok_i += 1
                        if sr > 0:
                            j0 = max(0, m0 - H)
                            if t == "u":
                                d = ss[q, j0:j0 + sr, :, :]
                            else:
                                d = ss[q, ns1 + j0:ns1 + j0 + sr, :, :]
                            oengs[ok_i % 2].dma_start(d, stage[su:su + sr, :])
                            ok_i += 1

    nc.finalize()
    return nc


def _get_nc(L):
    key = ("nc3", L)
    if key not in _CACHED:
        _CACHED[key] = _build_nc(L)
    return _CACHED[key]


def _ensure_trace_hook_safe():
    """If BASS_TRACE is set in the environment, run_bass_kernel_spmd imports
    antenv.axon_hooks, which may not exist. Install a working ctypes-based
    shim when possible, else disable tracing so the run cannot crash."""
    import os
    import sys
    import types

    if not os.environ.get("BASS_TRACE"):
        return
    try:
        import antenv.axon_hooks  # noqa: F401
        return
    except ImportError:
        pass
    try:
        from trn_agent_boot.trn_boot import _ntff_profile_via_ctypes
        hooks = types.ModuleType("antenv.axon_hooks")
        hook = _ntff_profile_via_ctypes("/opt/axon/libaxon_pjrt.so")
        hooks.get_axon_ntff_profile_hook = lambda: hook
        hooks.set_axon_ntff_profile_hook = lambda h: None
        sys.modules["antenv.axon_hooks"] = hooks
    except Exception:
        os.environ["BASS_NEVER_TRACE"] = "1"


def kernel(x: np.ndarray):
    from concourse.bass_utils import run_bass_kernel_spmd

    _ensure_trace_hook_safe()
    x = np.ascontiguousarray(np.asarray(x, dtype=np.float32))
    assert x.shape == (B, T, C)

    # ---- host: data-dependent truncation length L (tiny, exact math) ----
    M64 = _dct_mat(T)
    xbar = x.astype(np.float64).mean(axis=(0, 2))
    vq = np.abs(M64 @ xbar)
    thr = np.abs(np.quantile(vq, Q))
    idxs = np.where(vq > thr)[0]
    last_index = int(idxs[-1]) if idxs.size > 0 else -1
    L = last_index if last_index >= 0 else T - 1

    ns1 = (L + 1) // 2
    Wu, Wv = _build_weights(L)              # [H+ns1, 288], [H+ns2, 288]
    wu16 = np.ascontiguousarray(Wu.T).astype(np.float16)   # [288, H+ns1]
    wv16 = np.ascontiguousarray(Wv.T).astype(np.float16)

    # ---- host: fold input ----
    xf = x[:, :H, :]
    xr = x[:, T - 1:H - 1:-1, :]
    u16 = (xf + xr).astype(np.float16)
    v16 = (xf - xr).astype(np.float16)

    nc = _get_nc(L)

    def pack_x(z16):
        # [BPC,288,C] -> [2,2,128,4C] (q, ki, p, (b c)) + rem [2,128,C]
        full = z16[:, :256].reshape(2, 4, 2, 128, C)
        full = np.ascontiguousarray(full.transpose(0, 2, 3, 1, 4)
                                    ).reshape(2, 2, 128, 4 * C)
        remn = np.ascontiguousarray(z16[:, 256:288]).reshape(2, 128, C)
        return full, remn

    def pack_w(w16):
        # [288, M] -> [128, 2M] cols (ki m) + rem rows replicated [128, M]
        full = np.ascontiguousarray(w16[:256].reshape(2, 128, w16.shape[1])
                                    .transpose(1, 0, 2)
                                    ).reshape(128, 2 * w16.shape[1])
        remn = np.ascontiguousarray(np.tile(w16[256:288], (4, 1)))
        return full, remn

    wub_h, wur_h = pack_w(wu16)
    wvb_h, wvr_h = pack_w(wv16)
    in_maps = []
    for i in range(NCORES):
        xu_h, xur_h = pack_x(u16[i * BPC:(i + 1) * BPC])
        xv_h, xvr_h = pack_x(v16[i * BPC:(i + 1) * BPC])
        in_maps.append({"xu": xu_h, "xv": xv_h, "xur": xur_h, "xvr": xvr_h,
                        "wub": wub_h, "wvb": wvb_h,
                        "wur": wur_h, "wvr": wvr_h})
    res = run_bass_kernel_spmd(nc, in_maps, list(range(NCORES)))
    _CACHED["last_exec_time_ns"] = res.exec_time_ns

    # device layout [2, L, 4, C] (quad-major) -> [BPC, L, C] per core
    yy = np.concatenate(
        [res.results[i]["yy"].transpose(0, 2, 1, 3).reshape(BPC, L, C)
         for i in range(NCORES)], axis=0)
    ss = np.concatenate(
        [res.results[i]["ss"].transpose(0, 2, 1, 3).reshape(BPC, L, C)
         for i in range(NCORES)], axis=0)

    x_dct_trunc = np.empty((B, L, C), dtype=np.float32)
    x_dct_trunc[:, 0::2, :] = yy[:, :ns1, :].astype(np.float32)
    x_dct_trunc[:, 1::2, :] = yy[:, ns1:, :].astype(np.float32)
    a32 = ss[:, :ns1, :].astype(np.float32)
    b32 = ss[:, ns1:, :].astype(np.float32)
    ns2 = L // 2
    state = np.empty((B, L, C), dtype=np.float16)
    state[:, :ns2, :] = (a32[:, :ns2] + b32).astype(np.float16)
    if ns1 > ns2:
        state[:, ns2:ns1, :] = ss[:, ns2:ns1, :]   # lone middle row, L odd
    state[:, ns1:, :] = (a32[:, :ns2] - b32).astype(np.float16)[:, ::-1, :]
    return state, x_dct_trunc


# revision 17
# speedup vs baseline: 1.1780x; 1.0157x over previous
"""Trainium2 Bass kernel for DCTLAVISBlip dc_transform (DCT -> truncate -> IDCT).

Strategy (v3: v2 symmetry-folded matmuls + DMA/tail restructure)
----------------------------------------------------------------
Math identical to v2 (see kernel_v2_baseline.py): fold the input on the
host, run Wu = [Me; Pe'] and Wv = [Mo; Po'] ([575, 288] each) against
u/v in fp16, ship y and raw a/b state halves as f16, combine on host.

v3 changes (from the v2 trace: PE busy 92.4us of a 130us kernel, with a
~20us output-DMA tail and 10.5us of input-wait stalls at the head):
  1. Output staging batches 4 batches per tile: stage [mmt, 4C] per
     (q, t, m-tile), ONE dma_start per (stage, dest) -> 40 output DMA
     calls instead of 160.  DIRECT2D issue on the sequencers was
     0.6-3us per call; this kills most of the tail.
  2. Device DRAM output layout [2, L, 4, C] (quad-major) so each
     partition row ships 4 batches x 2KB = 8KB contiguous DRAM lines
     (4x fewer, 4x fatter descriptors).  Host transposes for free.
  3. Remainder (K=288 tail, 32 rows) matmuls issue FIRST in each wave:
     they only need the small rem input tiles, so the PE starts ~4us
     earlier, and each batch's PSUM accumulation completes right after
     its 2nd full matmul, spreading drains across the wave.
  4. Warmup starts immediately (memset on vector, 18 matmuls) so the
     HAM clock-gate window (~3.4us) is warm when real inputs land.
  5. Optional K-probe matmuls at the very end (PROBES flag) to measure
     matmul cost vs K for the next iteration's design.
"""

import numpy as np

B, T, C = 64, 576, 1024
H = T // 2                   # 288, folded K
NCORES = 8
BPC = B // NCORES            # batches per core
Q = 0.8
PROBES = True

_CACHED = {}


def _dct_mat(N):
    n = np.arange(N)
    Mm = np.cos(np.pi * (2 * n[None, :] + 1) * n[:, None] / (2 * N))
    s = np.full(N, np.sqrt(2.0 / N))
    s[0] = np.sqrt(1.0 / N)
    return s[:, None] * Mm          # float64


def _build_weights(L):
    """Wu [H+ns1, 288] = [Me; pad; Pe'], Wv [H+ns2, 288] = [Mo; pad; Po'].
    The y block is zero-padded up to H=288 rows so the state block starts at
    a 32-aligned PSUM partition in every m-tile."""
    M64 = _dct_mat(T)
    Mi = _dct_mat(L)
    ke = np.arange(0, L, 2)
    ko = np.arange(1, L, 2)
    Pe = np.einsum('kj,kt->jt', Mi[ke, :], M64[ke, :])
    Po = np.einsum('kj,kt->jt', Mi[ko, :], M64[ko, :])
    ns1 = (L + 1) // 2
    ns2 = L // 2
    pe_u = np.zeros((H - len(ke), H))
    pe_v = np.zeros((H - len(ko), H))
    Wu = np.concatenate([M64[ke][:, :H], pe_u, Pe[:ns1, :H]], axis=0)
    Wv = np.concatenate([M64[ko][:, :H], pe_v, Po[:ns2, :H]], axis=0)
    return Wu, Wv


def _build_nc(L):
    """Bass program for truncation length L (574 for the seed-0 input).

    Inputs host-packed as in v2:
      xu/xv  [2, 2, 128, 4C] f16: (q, ki, p, (b c))
      xur/xvr [2, 128, C]: K-remainder rows of 4 batches packed on partitions
      wub/wvb [128, 2M]: cols (ki m); wur/wvr [128, M]: rem rows 4x-replic.
    Outputs (v3): yy/ss [2, L, 4, C] f16 -- quad-major so one dma_start per
    (q, t, m-tile, dest) ships 4 batches with 8KB-contiguous DRAM lines.
    """
    import concourse.bacc as bacc
    import concourse.mybir as mybir
    import concourse.tile as tile

    f16 = mybir.dt.float16
    f32 = mybir.dt.float32

    ns1 = (L + 1) // 2
    ns2 = L // 2
    MU = H + ns1
    MV = H + ns2
    MW = {"u": MU, "v": MV}
    YB = {"u": ns1, "v": ns2}         # y rows per transform
    NT = [(0, 512), (512, 512)]
    MM = max(MU, MV)
    MT = [(m0, min(128, MM - m0)) for m0 in range(0, MM, 128)]

    f8 = mybir.dt.float8e3

    nc = bacc.Bacc("TRN2", target_bir_lowering=False, debug=False,
                   num_devices=NCORES)
    xu = nc.dram_tensor("xu", [2, 2, 128, 4 * C], f16, kind="ExternalInput")
    xv = nc.dram_tensor("xv", [2, 2, 128, 4 * C], f8, kind="ExternalInput")
    xur = nc.dram_tensor("xur", [2, 128, C], f16, kind="ExternalInput")
    xvr = nc.dram_tensor("xvr", [2, 128, C], f8, kind="ExternalInput")
    wub = nc.dram_tensor("wub", [128, 2 * MU], f16, kind="ExternalInput")
    wvb = nc.dram_tensor("wvb", [128, 2 * MV], f16, kind="ExternalInput")
    wur = nc.dram_tensor("wur", [128, MU], f16, kind="ExternalInput")
    wvr = nc.dram_tensor("wvr", [128, MV], f16, kind="ExternalInput")
    yy = nc.dram_tensor("yy", [2, L, 4, C], f16, kind="ExternalOutput")
    ss = nc.dram_tensor("ss", [2, L, 4, C], f16, kind="ExternalOutput")
    XD = {"u": (xu, xur, wub, wur), "v": (xv, xvr, wvb, wvr)}

    with tile.TileContext(nc) as tc:
        with (
            tc.tile_pool(name="wpool", bufs=1) as wpool,
            tc.tile_pool(name="xpool", bufs=1) as xpool,
            tc.tile_pool(name="opool", bufs=6) as opool,
            tc.tile_pool(name="ps", bufs=8, space="PSUM") as ps,
        ):
            # --- warmup immediately: memset on vector (idle at start) ---
            wz = wpool.tile([128, 128], f16, tag="wz", name="wz")
            nc.vector.memset(wz[:], 0.0)
            pwarm = ps.tile([128, 512], f32, tag="pt", name="pt")
            for _ in range(18):
                nc.tensor.matmul(pwarm[:, 0:128], wz[:], wz[:],
                                 start=True, stop=True)

            # --- input kicks, first-use order, spread across engines ---
            xt, rt, wt, wr = {}, {}, {}, {}

            def load_w(t, eng):
                _, _, wd, wrd = XD[t]
                w_ = wpool.tile([128, 2 * MW[t]], f16, tag=f"w{t}",
                                name=f"w{t}")
                eng.dma_start(w_[:], wd[:, :])
                wt[t] = w_
                w_ = wpool.tile([128, MW[t]], f16, tag=f"w{t}r",
                                name=f"w{t}r")
                eng.dma_start(w_[:], wrd[:, :])
                wr[t] = w_

            load_w("u", nc.scalar)
            load_w("v", nc.scalar)
            # inputs on the Act (scalar) HWDGE ring: the sync ring stays
            # free for output issue from the very first wave
            xdt = {"u": f16, "v": f8}
            for q in range(2):
                for t in ("u", "v"):
                    xd, rd, _, _ = XD[t]
                    for ki in range(2):
                        x_ = xpool.tile([128, 4 * C], xdt[t],
                                        tag=f"x{t}{q}{ki}",
                                        name=f"x{t}{q}{ki}")
                        nc.scalar.dma_start(x_[:, 0:2 * C],
                                            xd[q, ki, :, 0:2 * C])
                        xt[(t, q, ki)] = x_
                    r_ = xpool.tile([128, C], xdt[t], tag=f"x{t}r{q}",
                                    name=f"x{t}r{q}")
                    nc.gpsimd.dma_start(r_[:], rd[q, :, :])
                    rt[(t, q)] = r_
                    for ki in range(2):
                        nc.scalar.dma_start(xt[(t, q, ki)][:, 2 * C:4 * C],
                                            xd[q, ki, :, 2 * C:4 * C])

            def vcopy(dst, src):
                nc.vector.tensor_copy(dst, src)

            def scopy(dst, src):
                nc.scalar.copy(dst, src)

            oengs = [nc.sync, nc.gpsimd]
            ok_i = 0     # output call counter (engine rotation)
            pending = None   # delayed output call issued via scalar ring

            # --- compute waves: (q, t, m), 2 n-halves x 4 batches ---
            for q in range(2):
                for t in ("u", "v"):
                    mw = MW[t]
                    ybt = YB[t]
                    ykoff = 0 if t == "u" else ns1
                    for mi, (m0, mm) in enumerate(MT):
                        mmt = min(mm, mw - m0)
                        if mmt <= 0:
                            continue
                        yr = max(0, min(mmt, ybt - m0))   # y rows here
                        su = max(0, H - m0)               # state-local start
                        sr = max(0, mmt - su)             # state rows here
                        stage = opool.tile([128, 4 * C], f16,
                                           tag="o", name="o")
                        for ni, (n0, nn) in enumerate(NT):
                            # ramp: first m-tile of the run goes in 2-bank
                            # halves so compute starts on half the inputs
                            groups = ([(0, 1), (2, 3)]
                                      if (q == 0 and t == "u" and mi == 0)
                                      else [(0, 1, 2, 3)])
                            pts = {}
                            for grp in groups:
                                for bi in grp:
                                    pts[bi] = ps.tile([128, 512], f32,
                                                      tag="pt", name="pt")
                                for ki in range(2):
                                    wsl = wt[t][:, ki * mw + m0:
                                                ki * mw + m0 + mmt]
                                    for bi in grp:
                                        nc.tensor.matmul(
                                            pts[bi][0:mmt, :],
                                            wsl,
                                            xt[(t, q, ki)][:, bi * C + n0:
                                                           bi * C + n0 + nn],
                                            start=(ki == 0), stop=False)
                                for bi in grp:
                                    nc.tensor.matmul(
                                        pts[bi][0:mmt, :],
                                        wr[t][32 * bi:32 * bi + 32,
                                              m0:m0 + mmt],
                                        rt[(t, q)][32 * bi:32 * bi + 32,
                                                   n0:n0 + nn],
                                        start=False, stop=True,
                                        tile_position=(32 * bi, 0))
                            for bi in range(4):
                                cp = vcopy if bi % 2 == 0 else scopy
                                cp(stage[0:mmt, bi * C + n0:bi * C + n0 + nn],
                                   pts[bi][0:mmt, :])
                        # one dma_start per dest ships all 4 batches.
                        # y on sync/gpsimd immediately; state via the
                        # scalar (Act) ring delayed ONE wave so its issue
                        # never blocks this wave's scalar drains.
                        if pending is not None:
                            nc.scalar.dma_start(*pending)
                            pending = None
                        if yr > 0:
                            d = yy[q, ykoff + m0:ykoff + m0 + yr, :, :]
                            oengs[ok_i % 2].dma_start(d, stage[0:yr, :])
                            ok_i += 1
                        if sr > 0:
                            j0 = max(0, m0 - H)
                            if t == "u":
                                d = ss[q, j0:j0 + sr, :, :]
                            else:
                                d = ss[q, ns1 + j0:ns1 + j0 + sr, :, :]
                            pending = (d, stage[su:su + sr, :])
            if pending is not None:
                nc.scalar.dma_start(*pending)
                pending = None

    nc.finalize()
    return nc


def _get_nc(L):
    key = ("nc3", L)
    if key not in _CACHED:
        _CACHED[key] = _build_nc(L)
    return _CACHED[key]


def _ensure_trace_hook_safe():
    """If BASS_TRACE is set in the environment, run_bass_kernel_spmd imports
    antenv.axon_hooks, which may not exist. Install a working ctypes-based
    shim when possible, else disable tracing so the run cannot crash."""
    import os
    import sys
    import types

    if not os.environ.get("BASS_TRACE"):
        return
    try:
        import antenv.axon_hooks  # noqa: F401
        return
    except ImportError:
        pass
    try:
        from trn_agent_boot.trn_boot import _ntff_profile_via_ctypes
        hooks = types.ModuleType("antenv.axon_hooks")
        hook = _ntff_profile_via_ctypes("/opt/axon/libaxon_pjrt.so")
        hooks.get_axon_ntff_profile_hook = lambda: hook
        hooks.set_axon_ntff_profile_hook = lambda h: None
        sys.modules["antenv.axon_hooks"] = hooks
    except Exception:
        os.environ["BASS_NEVER_TRACE"] = "1"


def kernel(x: np.ndarray):
    from concourse.bass_utils import run_bass_kernel_spmd

    _ensure_trace_hook_safe()
    x = np.ascontiguousarray(np.asarray(x, dtype=np.float32))
    assert x.shape == (B, T, C)

    # ---- host: data-dependent truncation length L (tiny, exact math) ----
    M64 = _dct_mat(T)
    xbar = x.astype(np.float64).mean(axis=(0, 2))
    vq = np.abs(M64 @ xbar)
    thr = np.abs(np.quantile(vq, Q))
    idxs = np.where(vq > thr)[0]
    last_index = int(idxs[-1]) if idxs.size > 0 else -1
    L = last_index if last_index >= 0 else T - 1

    ns1 = (L + 1) // 2
    Wu, Wv = _build_weights(L)              # [H+ns1, 288], [H+ns2, 288]
    wu16 = np.ascontiguousarray(Wu.T).astype(np.float16)   # [288, H+ns1]
    wv16 = np.ascontiguousarray(Wv.T).astype(np.float16)

    # ---- host: fold input (u ships f16, v ships fp8 e3m4) ----
    import ml_dtypes
    xf = x[:, :H, :]
    xr = x[:, T - 1:H - 1:-1, :]
    u16 = (xf + xr).astype(np.float16)
    v16 = (xf - xr).astype(ml_dtypes.float8_e3m4)

    nc = _get_nc(L)

    def pack_x(z16):
        # [BPC,288,C] -> [2,2,128,4C] (q, ki, p, (b c)) + rem [2,128,C]
        full = z16[:, :256].reshape(2, 4, 2, 128, C)
        full = np.ascontiguousarray(full.transpose(0, 2, 3, 1, 4)
                                    ).reshape(2, 2, 128, 4 * C)
        remn = np.ascontiguousarray(z16[:, 256:288]).reshape(2, 128, C)
        return full, remn

    def pack_w(w16):
        # [288, M] -> [128, 2M] cols (ki m) + rem rows replicated [128, M]
        full = np.ascontiguousarray(w16[:256].reshape(2, 128, w16.shape[1])
                                    .transpose(1, 0, 2)
                                    ).reshape(128, 2 * w16.shape[1])
        remn = np.ascontiguousarray(np.tile(w16[256:288], (4, 1)))
        return full, remn

    wub_h, wur_h = pack_w(wu16)
    wvb_h, wvr_h = pack_w(wv16)
    in_maps = []
    for i in range(NCORES):
        xu_h, xur_h = pack_x(u16[i * BPC:(i + 1) * BPC])
        xv_h, xvr_h = pack_x(v16[i * BPC:(i + 1) * BPC])
        in_maps.append({"xu": xu_h, "xv": xv_h, "xur": xur_h, "xvr": xvr_h,
                        "wub": wub_h, "wvb": wvb_h,
                        "wur": wur_h, "wvr": wvr_h})
    res = run_bass_kernel_spmd(nc, in_maps, list(range(NCORES)))
    _CACHED["last_exec_time_ns"] = res.exec_time_ns

    # device layout [2, L, 4, C] (quad-major) -> [BPC, L, C] per core
    yy = np.concatenate(
        [res.results[i]["yy"].transpose(0, 2, 1, 3).reshape(BPC, L, C)
         for i in range(NCORES)], axis=0)
    ss = np.concatenate(
        [res.results[i]["ss"].transpose(0, 2, 1, 3).reshape(BPC, L, C)
         for i in range(NCORES)], axis=0)

    x_dct_trunc = np.empty((B, L, C), dtype=np.float32)
    x_dct_trunc[:, 0::2, :] = yy[:, :ns1, :].astype(np.float32)
    x_dct_trunc[:, 1::2, :] = yy[:, ns1:, :].astype(np.float32)
    a32 = ss[:, :ns1, :].astype(np.float32)
    b32 = ss[:, ns1:, :].astype(np.float32)
    ns2 = L // 2
    state = np.empty((B, L, C), dtype=np.float16)
    state[:, :ns2, :] = (a32[:, :ns2] + b32).astype(np.float16)
    if ns1 > ns2:
        state[:, ns2:ns1, :] = ss[:, ns2:ns1, :]   # lone middle row, L odd
    state[:, ns1:, :] = (a32[:, :ns2] - b32).astype(np.float16)[:, ::-1, :]
    return state, x_dct_trunc


# revision 24
# speedup vs baseline: 1.2340x; 1.0475x over previous
"""Trainium2 Bass kernel for DCTLAVISBlip dc_transform (DCT -> truncate -> IDCT).

Strategy (v3: v2 symmetry-folded matmuls + DMA/tail restructure)
----------------------------------------------------------------
Math identical to v2 (see kernel_v2_baseline.py): fold the input on the
host, run Wu = [Me; Pe'] and Wv = [Mo; Po'] ([575, 288] each) against
u/v in fp16, ship y and raw a/b state halves as f16, combine on host.

v3 changes (from the v2 trace: PE busy 92.4us of a 130us kernel, with a
~20us output-DMA tail and 10.5us of input-wait stalls at the head):
  1. Output staging batches 4 batches per tile: stage [mmt, 4C] per
     (q, t, m-tile), ONE dma_start per (stage, dest) -> 40 output DMA
     calls instead of 160.  DIRECT2D issue on the sequencers was
     0.6-3us per call; this kills most of the tail.
  2. Device DRAM output layout [2, L, 4, C] (quad-major) so each
     partition row ships 4 batches x 2KB = 8KB contiguous DRAM lines
     (4x fewer, 4x fatter descriptors).  Host transposes for free.
  3. Remainder (K=288 tail, 32 rows) matmuls issue FIRST in each wave:
     they only need the small rem input tiles, so the PE starts ~4us
     earlier, and each batch's PSUM accumulation completes right after
     its 2nd full matmul, spreading drains across the wave.
  4. Warmup starts immediately (memset on vector, 18 matmuls) so the
     HAM clock-gate window (~3.4us) is warm when real inputs land.
  5. Optional K-probe matmuls at the very end (PROBES flag) to measure
     matmul cost vs K for the next iteration's design.
"""

import numpy as np

B, T, C = 64, 576, 1024
H = T // 2                   # 288, folded K
NCORES = 8
BPC = B // NCORES            # batches per core
Q = 0.8
PROBES = True

_CACHED = {}


def _dct_mat(N):
    n = np.arange(N)
    Mm = np.cos(np.pi * (2 * n[None, :] + 1) * n[:, None] / (2 * N))
    s = np.full(N, np.sqrt(2.0 / N))
    s[0] = np.sqrt(1.0 / N)
    return s[:, None] * Mm          # float64


def _build_weights(L):
    """Wu [H+ns1, 288] = [Me; pad; Pe'], Wv [H+ns2, 288] = [Mo; pad; Po'].
    The y block is zero-padded up to H=288 rows so the state block starts at
    a 32-aligned PSUM partition in every m-tile."""
    M64 = _dct_mat(T)
    Mi = _dct_mat(L)
    ke = np.arange(0, L, 2)
    ko = np.arange(1, L, 2)
    Pe = np.einsum('kj,kt->jt', Mi[ke, :], M64[ke, :])
    Po = np.einsum('kj,kt->jt', Mi[ko, :], M64[ko, :])
    ns1 = (L + 1) // 2
    ns2 = L // 2
    pe_u = np.zeros((H - len(ke), H))
    pe_v = np.zeros((H - len(ko), H))
    Wu = np.concatenate([M64[ke][:, :H], pe_u, Pe[:ns1, :H]], axis=0)
    Wv = np.concatenate([M64[ko][:, :H], pe_v, Po[:ns2, :H]], axis=0)
    return Wu, Wv


def _build_nc(L):
    """Bass program for truncation length L (574 for the seed-0 input).

    Inputs host-packed as in v2:
      xu/xv  [2, 2, 128, 4C] f16: (q, ki, p, (b c))
      xur/xvr [2, 128, C]: K-remainder rows of 4 batches packed on partitions
      wub/wvb [128, 2M]: cols (ki m); wur/wvr [128, M]: rem rows 4x-replic.
    Outputs (v3): yy/ss [2, L, 4, C] f16 -- quad-major so one dma_start per
    (q, t, m-tile, dest) ships 4 batches with 8KB-contiguous DRAM lines.
    """
    import concourse.bacc as bacc
    import concourse.mybir as mybir
    import concourse.tile as tile

    f16 = mybir.dt.float16
    f32 = mybir.dt.float32

    ns1 = (L + 1) // 2
    ns2 = L // 2
    MU = H + ns1
    MV = H + ns2
    MW = {"u": MU, "v": MV}
    YB = {"u": ns1, "v": ns2}         # y rows per transform
    NT = [(0, 512), (512, 512)]
    MM = max(MU, MV)
    MT = [(m0, min(128, MM - m0)) for m0 in range(0, MM, 128)]

    f8 = mybir.dt.float8e3

    nc = bacc.Bacc("TRN2", target_bir_lowering=False, debug=False,
                   num_devices=NCORES)
    xu = nc.dram_tensor("xu", [2, 2, 128, 4 * C], f16, kind="ExternalInput")
    xv = nc.dram_tensor("xv", [2, 2, 128, 4 * C], f8, kind="ExternalInput")
    xur = nc.dram_tensor("xur", [2, 128, C], f16, kind="ExternalInput")
    xvr = nc.dram_tensor("xvr", [2, 128, C], f8, kind="ExternalInput")
    wub = nc.dram_tensor("wub", [128, 2 * MU], f16, kind="ExternalInput")
    wvb = nc.dram_tensor("wvb", [128, 2 * MV], f16, kind="ExternalInput")
    wur = nc.dram_tensor("wur", [128, MU], f16, kind="ExternalInput")
    wvr = nc.dram_tensor("wvr", [128, MV], f16, kind="ExternalInput")
    # one output tensor: plane t=0 holds [y-u rows; pad; a rows], t=1 holds
    # [y-v rows; pad; b rows] -- one dma_start per (q, t, m-tile)
    os_ = nc.dram_tensor("os", [2, 2, 576, 4 * C], f16,
                         kind="ExternalOutput")
    XD = {"u": (xu, xur, wub, wur), "v": (xv, xvr, wvb, wvr)}

    with tile.TileContext(nc) as tc:
        with (
            tc.tile_pool(name="wpool", bufs=1) as wpool,
            tc.tile_pool(name="xpool", bufs=1) as xpool,
            tc.tile_pool(name="opool", bufs=6) as opool,
            tc.tile_pool(name="ps", bufs=8, space="PSUM") as ps,
        ):
            # --- warmup immediately: memset on vector (idle at start) ---
            wz = wpool.tile([128, 128], f16, tag="wz", name="wz")
            nc.vector.memset(wz[:], 0.0)
            pwarm = ps.tile([128, 512], f32, tag="pt", name="pt")
            for _ in range(18):
                nc.tensor.matmul(pwarm[:, 0:128], wz[:], wz[:],
                                 start=True, stop=True)

            # --- input kicks, first-use order, spread across engines ---
            xt, rt, wt, wr = {}, {}, {}, {}

            def load_w(t, eng):
                _, _, wd, wrd = XD[t]
                w_ = wpool.tile([128, 2 * MW[t]], f16, tag=f"w{t}",
                                name=f"w{t}")
                eng.dma_start(w_[:], wd[:, :])
                wt[t] = w_
                w_ = wpool.tile([128, MW[t]], f16, tag=f"w{t}r",
                                name=f"w{t}r")
                eng.dma_start(w_[:], wrd[:, :])
                wr[t] = w_

            load_w("u", nc.sync)
            load_w("v", nc.sync)
            # inputs on the Act (scalar) HWDGE ring: the sync ring stays
            # free for output issue from the very first wave
            xdt = {"u": f16, "v": f8}
            for q in range(2):
                for t in ("u", "v"):
                    xd, rd, _, _ = XD[t]
                    for ki in range(2):
                        x_ = xpool.tile([128, 4 * C], xdt[t],
                                        tag=f"x{t}{q}{ki}",
                                        name=f"x{t}{q}{ki}")
                        nc.scalar.dma_start(x_[:, 0:2 * C],
                                            xd[q, ki, :, 0:2 * C])
                        xt[(t, q, ki)] = x_
                    r_ = xpool.tile([128, C], xdt[t], tag=f"x{t}r{q}",
                                    name=f"x{t}r{q}")
                    nc.gpsimd.dma_start(r_[:], rd[q, :, :])
                    rt[(t, q)] = r_
                    for ki in range(2):
                        nc.scalar.dma_start(xt[(t, q, ki)][:, 2 * C:4 * C],
                                            xd[q, ki, :, 2 * C:4 * C])

            def vcopy(dst, src):
                nc.vector.tensor_copy(dst, src)

            def scopy(dst, src):
                nc.scalar.copy(dst, src)

            oengs = [nc.sync, nc.gpsimd]
            ok_i = 0     # output call counter (engine rotation)
            pending = None   # delayed output call issued via scalar ring

            # --- compute waves: (q, t, m), 2 n-halves x 4 batches ---
            for q in range(2):
                for ti, t in enumerate(("u", "v")):
                    mw = MW[t]
                    for mi, (m0, mm) in enumerate(MT):
                        mmt = min(mm, mw - m0)
                        if mmt <= 0:
                            continue
                        stage = opool.tile([128, 4 * C], f16,
                                           tag="o", name="o")
                        for ni, (n0, nn) in enumerate(NT):
                            # ramp: first m-tile of the run goes in 2-bank
                            # halves so compute starts on half the inputs
                            groups = ([(0, 1), (2, 3)]
                                      if (q == 0 and t == "u" and mi == 0)
                                      else [(0, 1, 2, 3)])
                            pts = {}
                            for grp in groups:
                                for bi in grp:
                                    pts[bi] = ps.tile([128, 512], f32,
                                                      tag="pt", name="pt")
                                for ki in range(2):
                                    wsl = wt[t][:, ki * mw + m0:
                                                ki * mw + m0 + mmt]
                                    for bi in grp:
                                        nc.tensor.matmul(
                                            pts[bi][0:mmt, :],
                                            wsl,
                                            xt[(t, q, ki)][:, bi * C + n0:
                                                           bi * C + n0 + nn],
                                            start=(ki == 0), stop=False)
                                for bi in grp:
                                    nc.tensor.matmul(
                                        pts[bi][0:mmt, :],
                                        wr[t][32 * bi:32 * bi + 32,
                                              m0:m0 + mmt],
                                        rt[(t, q)][32 * bi:32 * bi + 32,
                                                   n0:n0 + nn],
                                        start=False, stop=True,
                                        tile_position=(32 * bi, 0))
                            for bi in range(4):
                                cp = vcopy if bi % 2 == 0 else scopy
                                cp(stage[0:mmt, bi * C + n0:bi * C + n0 + nn],
                                   pts[bi][0:mmt, :])
                        # ONE dma_start per wave ships y+state rows of all
                        # 4 batches.  Rotation sync/gpsimd immediate; every
                        # third call goes via the scalar (Act) ring delayed
                        # ONE wave so its issue never blocks scalar drains.
                        if pending is not None:
                            nc.scalar.dma_start(*pending)
                            pending = None
                        d = os_[q, ti, m0:m0 + mmt, :]
                        if ok_i % 3 == 2:
                            pending = (d, stage[0:mmt, :])
                        else:
                            oengs[ok_i % 3].dma_start(d, stage[0:mmt, :])
                        ok_i += 1
            if pending is not None:
                nc.scalar.dma_start(*pending)
                pending = None

    nc.finalize()
    return nc


def _get_nc(L):
    key = ("nc3", L)
    if key not in _CACHED:
        _CACHED[key] = _build_nc(L)
    return _CACHED[key]


def _ensure_trace_hook_safe():
    """If BASS_TRACE is set in the environment, run_bass_kernel_spmd imports
    antenv.axon_hooks, which may not exist. Install a working ctypes-based
    shim when possible, else disable tracing so the run cannot crash."""
    import os
    import sys
    import types

    if not os.environ.get("BASS_TRACE"):
        return
    try:
        import antenv.axon_hooks  # noqa: F401
        return
    except ImportError:
        pass
    try:
        from trn_agent_boot.trn_boot import _ntff_profile_via_ctypes
        hooks = types.ModuleType("antenv.axon_hooks")
        hook = _ntff_profile_via_ctypes("/opt/axon/libaxon_pjrt.so")
        hooks.get_axon_ntff_profile_hook = lambda: hook
        hooks.set_axon_ntff_profile_hook = lambda h: None
        sys.modules["antenv.axon_hooks"] = hooks
    except Exception:
        os.environ["BASS_NEVER_TRACE"] = "1"


def kernel(x: np.ndarray):
    from concourse.bass_utils import run_bass_kernel_spmd

    _ensure_trace_hook_safe()
    x = np.ascontiguousarray(np.asarray(x, dtype=np.float32))
    assert x.shape == (B, T, C)

    # ---- host: data-dependent truncation length L (tiny, exact math) ----
    M64 = _dct_mat(T)
    xbar = x.astype(np.float64).mean(axis=(0, 2))
    vq = np.abs(M64 @ xbar)
    thr = np.abs(np.quantile(vq, Q))
    idxs = np.where(vq > thr)[0]
    last_index = int(idxs[-1]) if idxs.size > 0 else -1
    L = last_index if last_index >= 0 else T - 1

    ns1 = (L + 1) // 2
    Wu, Wv = _build_weights(L)              # [H+ns1, 288], [H+ns2, 288]
    wu16 = np.ascontiguousarray(Wu.T).astype(np.float16)   # [288, H+ns1]
    wv16 = np.ascontiguousarray(Wv.T).astype(np.float16)

    # ---- host: fold input (u ships f16, v ships fp8 e3m4) ----
    import ml_dtypes
    xf = x[:, :H, :]
    xr = x[:, T - 1:H - 1:-1, :]
    u16 = (xf + xr).astype(np.float16)
    v16 = (xf - xr).astype(ml_dtypes.float8_e3m4)

    nc = _get_nc(L)

    def pack_x(z16):
        # [BPC,288,C] -> [2,2,128,4C] (q, ki, p, (b c)) + rem [2,128,C]
        full = z16[:, :256].reshape(2, 4, 2, 128, C)
        full = np.ascontiguousarray(full.transpose(0, 2, 3, 1, 4)
                                    ).reshape(2, 2, 128, 4 * C)
        remn = np.ascontiguousarray(z16[:, 256:288]).reshape(2, 128, C)
        return full, remn

    def pack_w(w16):
        # [288, M] -> [128, 2M] cols (ki m) + rem rows replicated [128, M]
        full = np.ascontiguousarray(w16[:256].reshape(2, 128, w16.shape[1])
                                    .transpose(1, 0, 2)
                                    ).reshape(128, 2 * w16.shape[1])
        remn = np.ascontiguousarray(np.tile(w16[256:288], (4, 1)))
        return full, remn

    wub_h, wur_h = pack_w(wu16)
    wvb_h, wvr_h = pack_w(wv16)
    in_maps = []
    for i in range(NCORES):
        xu_h, xur_h = pack_x(u16[i * BPC:(i + 1) * BPC])
        xv_h, xvr_h = pack_x(v16[i * BPC:(i + 1) * BPC])
        in_maps.append({"xu": xu_h, "xv": xv_h, "xur": xur_h, "xvr": xvr_h,
                        "wub": wub_h, "wvb": wvb_h,
                        "wur": wur_h, "wvr": wvr_h})
    res = run_bass_kernel_spmd(nc, in_maps, list(range(NCORES)))
    _CACHED["last_exec_time_ns"] = res.exec_time_ns

    # device layout os [2(q), 2(t), 576, 4, C]:
    #   t=0 rows [0:ns1]=y-even, [H:H+ns1]=a;  t=1 [0:ns2]=y-odd, [H:H+ns2]=b
    ns2 = L // 2

    def unq(o, tp, r0, rn):
        # [2, rn, 4, C] -> [BPC, rn, C]
        return o[:, tp, r0:r0 + rn, :].reshape(2, rn, 4, C) \
            .transpose(0, 2, 1, 3).reshape(BPC, rn, C)

    osr = [np.asarray(res.results[i]["os"]).reshape(2, 2, 576, 4, C)
           for i in range(NCORES)]
    ye = np.concatenate([unq(o, 0, 0, ns1) for o in osr], axis=0)
    yo = np.concatenate([unq(o, 1, 0, ns2) for o in osr], axis=0)
    aa = np.concatenate([unq(o, 0, H, ns1) for o in osr], axis=0)
    bb = np.concatenate([unq(o, 1, H, ns2) for o in osr], axis=0)

    x_dct_trunc = np.empty((B, L, C), dtype=np.float32)
    x_dct_trunc[:, 0::2, :] = ye.astype(np.float32)
    x_dct_trunc[:, 1::2, :] = yo.astype(np.float32)
    a32 = aa.astype(np.float32)
    b32 = bb.astype(np.float32)
    state = np.empty((B, L, C), dtype=np.float16)
    state[:, :ns2, :] = (a32[:, :ns2] + b32).astype(np.float16)
    if ns1 > ns2:
        state[:, ns2:ns1, :] = aa[:, ns2:ns1, :]   # lone middle row, L odd
    state[:, ns1:, :] = (a32[:, :ns2] - b32).astype(np.float16)[:, ::-1, :]
    return state, x_dct_trunc


# revision 28
# speedup vs baseline: 1.2566x; 1.0183x over previous
"""Trainium2 Bass kernel for DCTLAVISBlip dc_transform (DCT -> truncate -> IDCT).

Strategy (v3: v2 symmetry-folded matmuls + DMA/tail restructure)
----------------------------------------------------------------
Math identical to v2 (see kernel_v2_baseline.py): fold the input on the
host, run Wu = [Me; Pe'] and Wv = [Mo; Po'] ([575, 288] each) against
u/v in fp16, ship y and raw a/b state halves as f16, combine on host.

v3 changes (from the v2 trace: PE busy 92.4us of a 130us kernel, with a
~20us output-DMA tail and 10.5us of input-wait stalls at the head):
  1. Output staging batches 4 batches per tile: stage [mmt, 4C] per
     (q, t, m-tile), ONE dma_start per (stage, dest) -> 40 output DMA
     calls instead of 160.  DIRECT2D issue on the sequencers was
     0.6-3us per call; this kills most of the tail.
  2. Device DRAM output layout [2, L, 4, C] (quad-major) so each
     partition row ships 4 batches x 2KB = 8KB contiguous DRAM lines
     (4x fewer, 4x fatter descriptors).  Host transposes for free.
  3. Remainder (K=288 tail, 32 rows) matmuls issue FIRST in each wave:
     they only need the small rem input tiles, so the PE starts ~4us
     earlier, and each batch's PSUM accumulation completes right after
     its 2nd full matmul, spreading drains across the wave.
  4. Warmup starts immediately (memset on vector, 18 matmuls) so the
     HAM clock-gate window (~3.4us) is warm when real inputs land.
  5. Optional K-probe matmuls at the very end (PROBES flag) to measure
     matmul cost vs K for the next iteration's design.
"""

import numpy as np

B, T, C = 64, 576, 1024
H = T // 2                   # 288, folded K
NCORES = 8
BPC = B // NCORES            # batches per core
Q = 0.8
PROBES = True

_CACHED = {}


def _dct_mat(N):
    n = np.arange(N)
    Mm = np.cos(np.pi * (2 * n[None, :] + 1) * n[:, None] / (2 * N))
    s = np.full(N, np.sqrt(2.0 / N))
    s[0] = np.sqrt(1.0 / N)
    return s[:, None] * Mm          # float64


def _build_weights(L):
    """Wu [H+ns1, 288] = [Me; pad; Pe'], Wv [H+ns2, 288] = [Mo; pad; Po'].
    The y block is zero-padded up to H=288 rows so the state block starts at
    a 32-aligned PSUM partition in every m-tile."""
    M64 = _dct_mat(T)
    Mi = _dct_mat(L)
    ke = np.arange(0, L, 2)
    ko = np.arange(1, L, 2)
    Pe = np.einsum('kj,kt->jt', Mi[ke, :], M64[ke, :])
    Po = np.einsum('kj,kt->jt', Mi[ko, :], M64[ko, :])
    ns1 = (L + 1) // 2
    ns2 = L // 2
    pe_u = np.zeros((H - len(ke), H))
    pe_v = np.zeros((H - len(ko), H))
    Wu = np.concatenate([M64[ke][:, :H], pe_u, Pe[:ns1, :H]], axis=0)
    Wv = np.concatenate([M64[ko][:, :H], pe_v, Po[:ns2, :H]], axis=0)
    return Wu, Wv


def _build_nc(L):
    """Bass program for truncation length L (574 for the seed-0 input).

    Inputs host-packed as in v2:
      xu/xv  [2, 2, 128, 4C] f16: (q, ki, p, (b c))
      xur/xvr [2, 128, C]: K-remainder rows of 4 batches packed on partitions
      wub/wvb [128, 2M]: cols (ki m); wur/wvr [128, M]: rem rows 4x-replic.
    Outputs (v3): yy/ss [2, L, 4, C] f16 -- quad-major so one dma_start per
    (q, t, m-tile, dest) ships 4 batches with 8KB-contiguous DRAM lines.
    """
    import concourse.bacc as bacc
    import concourse.mybir as mybir
    import concourse.tile as tile

    f16 = mybir.dt.float16
    f32 = mybir.dt.float32

    ns1 = (L + 1) // 2
    ns2 = L // 2
    MU = H + ns1
    MV = H + ns2
    MW = {"u": MU, "v": MV}
    YB = {"u": ns1, "v": ns2}         # y rows per transform
    NT = [(0, 512), (512, 512)]
    MM = max(MU, MV)
    MT = [(m0, min(128, MM - m0)) for m0 in range(0, MM, 128)]

    f8 = mybir.dt.float8e3

    nc = bacc.Bacc("TRN2", target_bir_lowering=False, debug=False,
                   num_devices=NCORES)
    xu = nc.dram_tensor("xu", [2, 2, 128, 4 * C], f16, kind="ExternalInput")
    xv = nc.dram_tensor("xv", [2, 2, 128, 4 * C], f8, kind="ExternalInput")
    xur = nc.dram_tensor("xur", [2, 128, C], f16, kind="ExternalInput")
    xvr = nc.dram_tensor("xvr", [2, 128, C], f8, kind="ExternalInput")
    wub = nc.dram_tensor("wub", [128, 2 * MU], f16, kind="ExternalInput")
    wvb = nc.dram_tensor("wvb", [128, 2 * MV], f16, kind="ExternalInput")
    wur = nc.dram_tensor("wur", [128, MU], f16, kind="ExternalInput")
    wvr = nc.dram_tensor("wvr", [128, MV], f16, kind="ExternalInput")
    # one output tensor: plane t=0 holds [y-u rows; pad; a rows], t=1 holds
    # [y-v rows; pad; b rows] -- one dma_start per (q, t, m-tile)
    os_ = nc.dram_tensor("os", [2, 2, 576, 4 * C], f16,
                         kind="ExternalOutput")
    XD = {"u": (xu, xur, wub, wur), "v": (xv, xvr, wvb, wvr)}

    with tile.TileContext(nc) as tc:
        with (
            tc.tile_pool(name="wpool", bufs=1) as wpool,
            tc.tile_pool(name="xpool", bufs=1) as xpool,
            tc.tile_pool(name="opool", bufs=6) as opool,
            tc.tile_pool(name="ps", bufs=8, space="PSUM") as ps,
        ):
            # --- warmup immediately: memset on vector (idle at start) ---
            wz = wpool.tile([128, 128], f16, tag="wz", name="wz")
            nc.vector.memset(wz[:], 0.0)
            pwarm = ps.tile([128, 512], f32, tag="pt", name="pt")
            for _ in range(18):
                nc.tensor.matmul(pwarm[:, 0:128], wz[:], wz[:],
                                 start=True, stop=True)

            # --- input kicks, first-use order, spread across engines ---
            xt, rt, wt, wr = {}, {}, {}, {}

            def load_w(t, eng):
                _, _, wd, wrd = XD[t]
                w_ = wpool.tile([128, 2 * MW[t]], f16, tag=f"w{t}",
                                name=f"w{t}")
                eng.dma_start(w_[:], wd[:, :])
                wt[t] = w_
                w_ = wpool.tile([128, MW[t]], f16, tag=f"w{t}r",
                                name=f"w{t}r")
                eng.dma_start(w_[:], wrd[:, :])
                wr[t] = w_

            load_w("u", nc.sync)
            load_w("v", nc.sync)
            # inputs on the Act (scalar) HWDGE ring: the sync ring stays
            # free for output issue from the very first wave
            xdt = {"u": f16, "v": f8}
            for q in range(2):
                for t in ("v", "u"):
                    xd, rd, _, _ = XD[t]
                    for ki in range(2):
                        x_ = xpool.tile([128, 4 * C], xdt[t],
                                        tag=f"x{t}{q}{ki}",
                                        name=f"x{t}{q}{ki}")
                        nc.scalar.dma_start(x_[:, 0:2 * C],
                                            xd[q, ki, :, 0:2 * C])
                        xt[(t, q, ki)] = x_
                    r_ = xpool.tile([128, C], xdt[t], tag=f"x{t}r{q}",
                                    name=f"x{t}r{q}")
                    nc.gpsimd.dma_start(r_[:], rd[q, :, :])
                    rt[(t, q)] = r_
                    for ki in range(2):
                        nc.scalar.dma_start(xt[(t, q, ki)][:, 2 * C:4 * C],
                                            xd[q, ki, :, 2 * C:4 * C])

            def vcopy(dst, src):
                nc.vector.tensor_copy(dst, src)

            def scopy(dst, src):
                nc.scalar.copy(dst, src)

            oengs = [nc.sync, nc.gpsimd]
            ok_i = 0     # output call counter (engine rotation)
            pending = None   # delayed output call issued via scalar ring

            # --- compute waves: (q, t, m), 2 n-halves x 4 batches ---
            # v first: its fp8 kicks are half the bytes, so the first
            # wave's inputs land earliest
            NW = 20          # total waves
            for q in range(2):
                for t in ("v", "u"):
                    ti = 0 if t == "u" else 1
                    mw = MW[t]
                    for mi, (m0, mm) in enumerate(MT):
                        mmt = min(mm, mw - m0)
                        if mmt <= 0:
                            continue
                        stage = opool.tile([128, 4 * C], f16,
                                           tag="o", name="o")
                        for ni, (n0, nn) in enumerate(NT):
                            # ramp: first m-tile of the run goes in 2-bank
                            # halves so compute starts on half the inputs
                            groups = ([(0, 1), (2, 3)]
                                      if (q == 0 and t == "v" and mi == 0)
                                      else [(0, 1, 2, 3)])
                            pts = {}
                            for grp in groups:
                                for bi in grp:
                                    pts[bi] = ps.tile([128, 512], f32,
                                                      tag="pt", name="pt")
                                for ki in range(2):
                                    wsl = wt[t][:, ki * mw + m0:
                                                ki * mw + m0 + mmt]
                                    for bi in grp:
                                        nc.tensor.matmul(
                                            pts[bi][0:mmt, :],
                                            wsl,
                                            xt[(t, q, ki)][:, bi * C + n0:
                                                           bi * C + n0 + nn],
                                            start=(ki == 0), stop=False)
                                for bi in grp:
                                    nc.tensor.matmul(
                                        pts[bi][0:mmt, :],
                                        wr[t][32 * bi:32 * bi + 32,
                                              m0:m0 + mmt],
                                        rt[(t, q)][32 * bi:32 * bi + 32,
                                                   n0:n0 + nn],
                                        start=False, stop=True,
                                        tile_position=(32 * bi, 0))
                            for bi in range(4):
                                cp = vcopy if bi % 2 == 0 else scopy
                                cp(stage[0:mmt, bi * C + n0:bi * C + n0 + nn],
                                   pts[bi][0:mmt, :])
                        # ONE dma_start per wave ships y+state rows of all
                        # 4 batches.  Rotation sync/gpsimd immediate; every
                        # third call goes via the scalar (Act) ring delayed
                        # ONE wave so its issue never blocks scalar drains.
                        if pending is not None:
                            nc.scalar.dma_start(*pending)
                            pending = None
                        d = os_[q, ti, m0:m0 + mmt, :]
                        if ok_i >= NW - 3:
                            # tail waves: split in column halves across the
                            # two free rings so the drain finishes fast
                            oengs[0].dma_start(d[:, 0:2 * C],
                                               stage[0:mmt, 0:2 * C])
                            oengs[1].dma_start(d[:, 2 * C:4 * C],
                                               stage[0:mmt, 2 * C:4 * C])
                        elif ok_i % 3 == 2:
                            pending = (d, stage[0:mmt, :])
                        else:
                            oengs[ok_i % 3].dma_start(d, stage[0:mmt, :])
                        ok_i += 1
            if pending is not None:
                nc.scalar.dma_start(*pending)
                pending = None

    nc.finalize()
    return nc


def _get_nc(L):
    key = ("nc3", L)
    if key not in _CACHED:
        _CACHED[key] = _build_nc(L)
    return _CACHED[key]


def _ensure_trace_hook_safe():
    """If BASS_TRACE is set in the environment, run_bass_kernel_spmd imports
    antenv.axon_hooks, which may not exist. Install a working ctypes-based
    shim when possible, else disable tracing so the run cannot crash."""
    import os
    import sys
    import types

    if not os.environ.get("BASS_TRACE"):
        return
    try:
        import antenv.axon_hooks  # noqa: F401
        return
    except ImportError:
        pass
    try:
        from trn_agent_boot.trn_boot import _ntff_profile_via_ctypes
        hooks = types.ModuleType("antenv.axon_hooks")
        hook = _ntff_profile_via_ctypes("/opt/axon/libaxon_pjrt.so")
        hooks.get_axon_ntff_profile_hook = lambda: hook
        hooks.set_axon_ntff_profile_hook = lambda h: None
        sys.modules["antenv.axon_hooks"] = hooks
    except Exception:
        os.environ["BASS_NEVER_TRACE"] = "1"


def kernel(x: np.ndarray):
    from concourse.bass_utils import run_bass_kernel_spmd

    _ensure_trace_hook_safe()
    x = np.ascontiguousarray(np.asarray(x, dtype=np.float32))
    assert x.shape == (B, T, C)

    # ---- host: data-dependent truncation length L (tiny, exact math) ----
    M64 = _dct_mat(T)
    xbar = x.astype(np.float64).mean(axis=(0, 2))
    vq = np.abs(M64 @ xbar)
    thr = np.abs(np.quantile(vq, Q))
    idxs = np.where(vq > thr)[0]
    last_index = int(idxs[-1]) if idxs.size > 0 else -1
    L = last_index if last_index >= 0 else T - 1

    ns1 = (L + 1) // 2
    Wu, Wv = _build_weights(L)              # [H+ns1, 288], [H+ns2, 288]
    wu16 = np.ascontiguousarray(Wu.T).astype(np.float16)   # [288, H+ns1]
    wv16 = np.ascontiguousarray(Wv.T).astype(np.float16)

    # ---- host: fold input (u ships f16, v ships fp8 e3m4) ----
    import ml_dtypes
    xf = x[:, :H, :]
    xr = x[:, T - 1:H - 1:-1, :]
    u16 = (xf + xr).astype(np.float16)
    v16 = (xf - xr).astype(ml_dtypes.float8_e3m4)

    nc = _get_nc(L)

    def pack_x(z16):
        # [BPC,288,C] -> [2,2,128,4C] (q, ki, p, (b c)) + rem [2,128,C]
        full = z16[:, :256].reshape(2, 4, 2, 128, C)
        full = np.ascontiguousarray(full.transpose(0, 2, 3, 1, 4)
                                    ).reshape(2, 2, 128, 4 * C)
        remn = np.ascontiguousarray(z16[:, 256:288]).reshape(2, 128, C)
        return full, remn

    def pack_w(w16):
        # [288, M] -> [128, 2M] cols (ki m) + rem rows replicated [128, M]
        full = np.ascontiguousarray(w16[:256].reshape(2, 128, w16.shape[1])
                                    .transpose(1, 0, 2)
                                    ).reshape(128, 2 * w16.shape[1])
        remn = np.ascontiguousarray(np.tile(w16[256:288], (4, 1)))
        return full, remn

    wub_h, wur_h = pack_w(wu16)
    wvb_h, wvr_h = pack_w(wv16)
    in_maps = []
    for i in range(NCORES):
        xu_h, xur_h = pack_x(u16[i * BPC:(i + 1) * BPC])
        xv_h, xvr_h = pack_x(v16[i * BPC:(i + 1) * BPC])
        in_maps.append({"xu": xu_h, "xv": xv_h, "xur": xur_h, "xvr": xvr_h,
                        "wub": wub_h, "wvb": wvb_h,
                        "wur": wur_h, "wvr": wvr_h})
    res = run_bass_kernel_spmd(nc, in_maps, list(range(NCORES)))
    _CACHED["last_exec_time_ns"] = res.exec_time_ns

    # device layout os [2(q), 2(t), 576, 4, C]:
    #   t=0 rows [0:ns1]=y-even, [H:H+ns1]=a;  t=1 [0:ns2]=y-odd, [H:H+ns2]=b
    ns2 = L // 2

    def unq(o, tp, r0, rn):
        # [2, rn, 4, C] -> [BPC, rn, C]
        return o[:, tp, r0:r0 + rn, :].reshape(2, rn, 4, C) \
            .transpose(0, 2, 1, 3).reshape(BPC, rn, C)

    osr = [np.asarray(res.results[i]["os"]).reshape(2, 2, 576, 4, C)
           for i in range(NCORES)]
    ye = np.concatenate([unq(o, 0, 0, ns1) for o in osr], axis=0)
    yo = np.concatenate([unq(o, 1, 0, ns2) for o in osr], axis=0)
    aa = np.concatenate([unq(o, 0, H, ns1) for o in osr], axis=0)
    bb = np.concatenate([unq(o, 1, H, ns2) for o in osr], axis=0)

    x_dct_trunc = np.empty((B, L, C), dtype=np.float32)
    x_dct_trunc[:, 0::2, :] = ye.astype(np.float32)
    x_dct_trunc[:, 1::2, :] = yo.astype(np.float32)
    a32 = aa.astype(np.float32)
    b32 = bb.astype(np.float32)
    state = np.empty((B, L, C), dtype=np.float16)
    state[:, :ns2, :] = (a32[:, :ns2] + b32).astype(np.float16)
    if ns1 > ns2:
        state[:, ns2:ns1, :] = aa[:, ns2:ns1, :]   # lone middle row, L odd
    state[:, ns1:, :] = (a32[:, :ns2] - b32).astype(np.float16)[:, ::-1, :]
    return state, x_dct_trunc


# revision 29
# speedup vs baseline: 1.2679x; 1.0090x over previous
"""Trainium2 Bass kernel for DCTLAVISBlip dc_transform (DCT -> truncate -> IDCT).

Strategy (v3: v2 symmetry-folded matmuls + DMA/tail restructure)
----------------------------------------------------------------
Math identical to v2 (see kernel_v2_baseline.py): fold the input on the
host, run Wu = [Me; Pe'] and Wv = [Mo; Po'] ([575, 288] each) against
u/v in fp16, ship y and raw a/b state halves as f16, combine on host.

v3 changes (from the v2 trace: PE busy 92.4us of a 130us kernel, with a
~20us output-DMA tail and 10.5us of input-wait stalls at the head):
  1. Output staging batches 4 batches per tile: stage [mmt, 4C] per
     (q, t, m-tile), ONE dma_start per (stage, dest) -> 40 output DMA
     calls instead of 160.  DIRECT2D issue on the sequencers was
     0.6-3us per call; this kills most of the tail.
  2. Device DRAM output layout [2, L, 4, C] (quad-major) so each
     partition row ships 4 batches x 2KB = 8KB contiguous DRAM lines
     (4x fewer, 4x fatter descriptors).  Host transposes for free.
  3. Remainder (K=288 tail, 32 rows) matmuls issue FIRST in each wave:
     they only need the small rem input tiles, so the PE starts ~4us
     earlier, and each batch's PSUM accumulation completes right after
     its 2nd full matmul, spreading drains across the wave.
  4. Warmup starts immediately (memset on vector, 18 matmuls) so the
     HAM clock-gate window (~3.4us) is warm when real inputs land.
  5. Optional K-probe matmuls at the very end (PROBES flag) to measure
     matmul cost vs K for the next iteration's design.
"""

import numpy as np

B, T, C = 64, 576, 1024
H = T // 2                   # 288, folded K
NCORES = 8
BPC = B // NCORES            # batches per core
Q = 0.8
PROBES = True

_CACHED = {}


def _dct_mat(N):
    n = np.arange(N)
    Mm = np.cos(np.pi * (2 * n[None, :] + 1) * n[:, None] / (2 * N))
    s = np.full(N, np.sqrt(2.0 / N))
    s[0] = np.sqrt(1.0 / N)
    return s[:, None] * Mm          # float64


def _build_weights(L):
    """Wu [H+ns1, 288] = [Me; pad; Pe'], Wv [H+ns2, 288] = [Mo; pad; Po'].
    The y block is zero-padded up to H=288 rows so the state block starts at
    a 32-aligned PSUM partition in every m-tile."""
    M64 = _dct_mat(T)
    Mi = _dct_mat(L)
    ke = np.arange(0, L, 2)
    ko = np.arange(1, L, 2)
    Pe = np.einsum('kj,kt->jt', Mi[ke, :], M64[ke, :])
    Po = np.einsum('kj,kt->jt', Mi[ko, :], M64[ko, :])
    ns1 = (L + 1) // 2
    ns2 = L // 2
    pe_u = np.zeros((H - len(ke), H))
    pe_v = np.zeros((H - len(ko), H))
    Wu = np.concatenate([M64[ke][:, :H], pe_u, Pe[:ns1, :H]], axis=0)
    Wv = np.concatenate([M64[ko][:, :H], pe_v, Po[:ns2, :H]], axis=0)
    return Wu, Wv


def _build_nc(L):
    """Bass program for truncation length L (574 for the seed-0 input).

    Inputs host-packed as in v2:
      xu/xv  [2, 2, 128, 4C] f16: (q, ki, p, (b c))
      xur/xvr [2, 128, C]: K-remainder rows of 4 batches packed on partitions
      wub/wvb [128, 2M]: cols (ki m); wur/wvr [128, M]: rem rows 4x-replic.
    Outputs (v3): yy/ss [2, L, 4, C] f16 -- quad-major so one dma_start per
    (q, t, m-tile, dest) ships 4 batches with 8KB-contiguous DRAM lines.
    """
    import concourse.bacc as bacc
    import concourse.mybir as mybir
    import concourse.tile as tile

    f16 = mybir.dt.float16
    f32 = mybir.dt.float32

    ns1 = (L + 1) // 2
    ns2 = L // 2
    MU = H + ns1
    MV = H + ns2
    MW = {"u": MU, "v": MV}
    YB = {"u": ns1, "v": ns2}         # y rows per transform
    NT = [(0, 512), (512, 512)]
    MM = max(MU, MV)
    MT = [(m0, min(128, MM - m0)) for m0 in range(0, MM, 128)]

    f8 = mybir.dt.float8e3

    nc = bacc.Bacc("TRN2", target_bir_lowering=False, debug=False,
                   num_devices=NCORES)
    xu = nc.dram_tensor("xu", [2, 2, 128, 4 * C], f16, kind="ExternalInput")
    xv = nc.dram_tensor("xv", [2, 2, 128, 4 * C], f8, kind="ExternalInput")
    xur = nc.dram_tensor("xur", [2, 128, C], f16, kind="ExternalInput")
    xvr = nc.dram_tensor("xvr", [2, 128, C], f8, kind="ExternalInput")
    wub = nc.dram_tensor("wub", [128, 2 * MU], f16, kind="ExternalInput")
    wvb = nc.dram_tensor("wvb", [128, 2 * MV], f16, kind="ExternalInput")
    wur = nc.dram_tensor("wur", [128, MU], f16, kind="ExternalInput")
    wvr = nc.dram_tensor("wvr", [128, MV], f16, kind="ExternalInput")
    # one output tensor: plane t=0 holds [y-u rows; pad; a rows], t=1 holds
    # [y-v rows; pad; b rows] -- one dma_start per (q, t, m-tile)
    os_ = nc.dram_tensor("os", [2, 2, 576, 4 * C], f16,
                         kind="ExternalOutput")
    XD = {"u": (xu, xur, wub, wur), "v": (xv, xvr, wvb, wvr)}

    with tile.TileContext(nc) as tc:
        with (
            tc.tile_pool(name="wpool", bufs=1) as wpool,
            tc.tile_pool(name="xpool", bufs=1) as xpool,
            tc.tile_pool(name="opool", bufs=6) as opool,
            tc.tile_pool(name="ps", bufs=8, space="PSUM") as ps,
        ):
            # --- warmup immediately: memset on vector (idle at start) ---
            wz = wpool.tile([128, 128], f16, tag="wz", name="wz")
            nc.vector.memset(wz[:], 0.0)
            pwarm = ps.tile([128, 512], f32, tag="pt", name="pt")
            for _ in range(18):
                nc.tensor.matmul(pwarm[:, 0:128], wz[:], wz[:],
                                 start=True, stop=True)

            # --- input kicks, first-use order, spread across engines ---
            xt, rt, wt, wr = {}, {}, {}, {}

            def load_w(t, eng):
                _, _, wd, wrd = XD[t]
                w_ = wpool.tile([128, 2 * MW[t]], f16, tag=f"w{t}",
                                name=f"w{t}")
                eng.dma_start(w_[:], wd[:, :])
                wt[t] = w_
                w_ = wpool.tile([128, MW[t]], f16, tag=f"w{t}r",
                                name=f"w{t}r")
                eng.dma_start(w_[:], wrd[:, :])
                wr[t] = w_

            load_w("v", nc.sync)     # v runs first
            load_w("u", nc.sync)
            # inputs on the Act (scalar) HWDGE ring: the sync ring stays
            # free for output issue from the very first wave
            xdt = {"u": f16, "v": f8}
            for q in range(2):
                for t in ("v", "u"):
                    xd, rd, _, _ = XD[t]
                    for ki in range(2):
                        x_ = xpool.tile([128, 4 * C], xdt[t],
                                        tag=f"x{t}{q}{ki}",
                                        name=f"x{t}{q}{ki}")
                        nc.scalar.dma_start(x_[:, 0:2 * C],
                                            xd[q, ki, :, 0:2 * C])
                        xt[(t, q, ki)] = x_
                    r_ = xpool.tile([128, C], xdt[t], tag=f"x{t}r{q}",
                                    name=f"x{t}r{q}")
                    nc.gpsimd.dma_start(r_[:], rd[q, :, :])
                    rt[(t, q)] = r_
                    for ki in range(2):
                        nc.scalar.dma_start(xt[(t, q, ki)][:, 2 * C:4 * C],
                                            xd[q, ki, :, 2 * C:4 * C])

            def vcopy(dst, src):
                nc.vector.tensor_copy(dst, src)

            def scopy(dst, src):
                nc.scalar.copy(dst, src)

            oengs = [nc.sync, nc.gpsimd]
            ok_i = 0     # output call counter (engine rotation)
            pending = None   # delayed output call issued via scalar ring

            # --- compute waves: (q, t, m), 2 n-halves x 4 batches ---
            # v first: its fp8 kicks are half the bytes, so the first
            # wave's inputs land earliest
            NW = 20          # total waves
            for q in range(2):
                for t in ("v", "u"):
                    ti = 0 if t == "u" else 1
                    mw = MW[t]
                    for mi, (m0, mm) in enumerate(MT):
                        mmt = min(mm, mw - m0)
                        if mmt <= 0:
                            continue
                        stage = opool.tile([128, 4 * C], f16,
                                           tag="o", name="o")
                        for ni, (n0, nn) in enumerate(NT):
                            # ramp: first m-tile of the run goes in 2-bank
                            # halves so compute starts on half the inputs
                            groups = ([(0, 1), (2, 3)]
                                      if (q == 0 and t == "v" and mi == 0)
                                      else [(0, 1, 2, 3)])
                            pts = {}
                            for grp in groups:
                                for bi in grp:
                                    pts[bi] = ps.tile([128, 512], f32,
                                                      tag="pt", name="pt")
                                for ki in range(2):
                                    wsl = wt[t][:, ki * mw + m0:
                                                ki * mw + m0 + mmt]
                                    for bi in grp:
                                        nc.tensor.matmul(
                                            pts[bi][0:mmt, :],
                                            wsl,
                                            xt[(t, q, ki)][:, bi * C + n0:
                                                           bi * C + n0 + nn],
                                            start=(ki == 0), stop=False)
                                for bi in grp:
                                    nc.tensor.matmul(
                                        pts[bi][0:mmt, :],
                                        wr[t][32 * bi:32 * bi + 32,
                                              m0:m0 + mmt],
                                        rt[(t, q)][32 * bi:32 * bi + 32,
                                                   n0:n0 + nn],
                                        start=False, stop=True,
                                        tile_position=(32 * bi, 0))
                            for bi in range(4):
                                cp = vcopy if bi % 2 == 0 else scopy
                                cp(stage[0:mmt, bi * C + n0:bi * C + n0 + nn],
                                   pts[bi][0:mmt, :])
                        # ONE dma_start per wave ships y+state rows of all
                        # 4 batches.  Rotation sync/gpsimd immediate; every
                        # third call goes via the scalar (Act) ring delayed
                        # ONE wave so its issue never blocks scalar drains.
                        if pending is not None:
                            nc.scalar.dma_start(*pending)
                            pending = None
                        d = os_[q, ti, m0:m0 + mmt, :]
                        if ok_i >= NW - 3:
                            # tail waves: split in column halves across the
                            # two free rings so the drain finishes fast
                            oengs[0].dma_start(d[:, 0:2 * C],
                                               stage[0:mmt, 0:2 * C])
                            oengs[1].dma_start(d[:, 2 * C:4 * C],
                                               stage[0:mmt, 2 * C:4 * C])
                        elif ok_i % 3 == 2:
                            pending = (d, stage[0:mmt, :])
                        else:
                            oengs[ok_i % 3].dma_start(d, stage[0:mmt, :])
                        ok_i += 1
            if pending is not None:
                nc.scalar.dma_start(*pending)
                pending = None

    nc.finalize()
    return nc


def _get_nc(L):
    key = ("nc3", L)
    if key not in _CACHED:
        _CACHED[key] = _build_nc(L)
    return _CACHED[key]


def _ensure_trace_hook_safe():
    """If BASS_TRACE is set in the environment, run_bass_kernel_spmd imports
    antenv.axon_hooks, which may not exist. Install a working ctypes-based
    shim when possible, else disable tracing so the run cannot crash."""
    import os
    import sys
    import types

    if not os.environ.get("BASS_TRACE"):
        return
    try:
        import antenv.axon_hooks  # noqa: F401
        return
    except ImportError:
        pass
    try:
        from trn_agent_boot.trn_boot import _ntff_profile_via_ctypes
        hooks = types.ModuleType("antenv.axon_hooks")
        hook = _ntff_profile_via_ctypes("/opt/axon/libaxon_pjrt.so")
        hooks.get_axon_ntff_profile_hook = lambda: hook
        hooks.set_axon_ntff_profile_hook = lambda h: None
        sys.modules["antenv.axon_hooks"] = hooks
    except Exception:
        os.environ["BASS_NEVER_TRACE"] = "1"


def kernel(x: np.ndarray):
    from concourse.bass_utils import run_bass_kernel_spmd

    _ensure_trace_hook_safe()
    x = np.ascontiguousarray(np.asarray(x, dtype=np.float32))
    assert x.shape == (B, T, C)

    # ---- host: data-dependent truncation length L (tiny, exact math) ----
    M64 = _dct_mat(T)
    xbar = x.astype(np.float64).mean(axis=(0, 2))
    vq = np.abs(M64 @ xbar)
    thr = np.abs(np.quantile(vq, Q))
    idxs = np.where(vq > thr)[0]
    last_index = int(idxs[-1]) if idxs.size > 0 else -1
    L = last_index if last_index >= 0 else T - 1

    ns1 = (L + 1) // 2
    Wu, Wv = _build_weights(L)              # [H+ns1, 288], [H+ns2, 288]
    wu16 = np.ascontiguousarray(Wu.T).astype(np.float16)   # [288, H+ns1]
    wv16 = np.ascontiguousarray(Wv.T).astype(np.float16)

    # ---- host: fold input (u ships f16, v ships fp8 e3m4) ----
    import ml_dtypes
    xf = x[:, :H, :]
    xr = x[:, T - 1:H - 1:-1, :]
    u16 = (xf + xr).astype(np.float16)
    v16 = (xf - xr).astype(ml_dtypes.float8_e3m4)

    nc = _get_nc(L)

    def pack_x(z16):
        # [BPC,288,C] -> [2,2,128,4C] (q, ki, p, (b c)) + rem [2,128,C]
        full = z16[:, :256].reshape(2, 4, 2, 128, C)
        full = np.ascontiguousarray(full.transpose(0, 2, 3, 1, 4)
                                    ).reshape(2, 2, 128, 4 * C)
        remn = np.ascontiguousarray(z16[:, 256:288]).reshape(2, 128, C)
        return full, remn

    def pack_w(w16):
        # [288, M] -> [128, 2M] cols (ki m) + rem rows replicated [128, M]
        full = np.ascontiguousarray(w16[:256].reshape(2, 128, w16.shape[1])
                                    .transpose(1, 0, 2)
                                    ).reshape(128, 2 * w16.shape[1])
        remn = np.ascontiguousarray(np.tile(w16[256:288], (4, 1)))
        return full, remn

    wub_h, wur_h = pack_w(wu16)
    wvb_h, wvr_h = pack_w(wv16)
    in_maps = []
    for i in range(NCORES):
        xu_h, xur_h = pack_x(u16[i * BPC:(i + 1) * BPC])
        xv_h, xvr_h = pack_x(v16[i * BPC:(i + 1) * BPC])
        in_maps.append({"xu": xu_h, "xv": xv_h, "xur": xur_h, "xvr": xvr_h,
                        "wub": wub_h, "wvb": wvb_h,
                        "wur": wur_h, "wvr": wvr_h})
    res = run_bass_kernel_spmd(nc, in_maps, list(range(NCORES)))
    _CACHED["last_exec_time_ns"] = res.exec_time_ns

    # device layout os [2(q), 2(t), 576, 4, C]:
    #   t=0 rows [0:ns1]=y-even, [H:H+ns1]=a;  t=1 [0:ns2]=y-odd, [H:H+ns2]=b
    ns2 = L // 2

    def unq(o, tp, r0, rn):
        # [2, rn, 4, C] -> [BPC, rn, C]
        return o[:, tp, r0:r0 + rn, :].reshape(2, rn, 4, C) \
            .transpose(0, 2, 1, 3).reshape(BPC, rn, C)

    osr = [np.asarray(res.results[i]["os"]).reshape(2, 2, 576, 4, C)
           for i in range(NCORES)]
    ye = np.concatenate([unq(o, 0, 0, ns1) for o in osr], axis=0)
    yo = np.concatenate([unq(o, 1, 0, ns2) for o in osr], axis=0)
    aa = np.concatenate([unq(o, 0, H, ns1) for o in osr], axis=0)
    bb = np.concatenate([unq(o, 1, H, ns2) for o in osr], axis=0)

    x_dct_trunc = np.empty((B, L, C), dtype=np.float32)
    x_dct_trunc[:, 0::2, :] = ye.astype(np.float32)
    x_dct_trunc[:, 1::2, :] = yo.astype(np.float32)
    a32 = aa.astype(np.float32)
    b32 = bb.astype(np.float32)
    state = np.empty((B, L, C), dtype=np.float16)
    state[:, :ns2, :] = (a32[:, :ns2] + b32).astype(np.float16)
    if ns1 > ns2:
        state[:, ns2:ns1, :] = aa[:, ns2:ns1, :]   # lone middle row, L odd
    state[:, ns1:, :] = (a32[:, :ns2] - b32).astype(np.float16)[:, ::-1, :]
    return state, x_dct_trunc


# revision 32
# speedup vs baseline: 1.2697x; 1.0014x over previous
"""Trainium2 Bass kernel for DCTLAVISBlip dc_transform (DCT -> truncate -> IDCT).

Strategy (v3: v2 symmetry-folded matmuls + DMA/tail restructure)
----------------------------------------------------------------
Math identical to v2 (see kernel_v2_baseline.py): fold the input on the
host, run Wu = [Me; Pe'] and Wv = [Mo; Po'] ([575, 288] each) against
u/v in fp16, ship y and raw a/b state halves as f16, combine on host.

v3 changes (from the v2 trace: PE busy 92.4us of a 130us kernel, with a
~20us output-DMA tail and 10.5us of input-wait stalls at the head):
  1. Output staging batches 4 batches per tile: stage [mmt, 4C] per
     (q, t, m-tile), ONE dma_start per (stage, dest) -> 40 output DMA
     calls instead of 160.  DIRECT2D issue on the sequencers was
     0.6-3us per call; this kills most of the tail.
  2. Device DRAM output layout [2, L, 4, C] (quad-major) so each
     partition row ships 4 batches x 2KB = 8KB contiguous DRAM lines
     (4x fewer, 4x fatter descriptors).  Host transposes for free.
  3. Remainder (K=288 tail, 32 rows) matmuls issue FIRST in each wave:
     they only need the small rem input tiles, so the PE starts ~4us
     earlier, and each batch's PSUM accumulation completes right after
     its 2nd full matmul, spreading drains across the wave.
  4. Warmup starts immediately (memset on vector, 18 matmuls) so the
     HAM clock-gate window (~3.4us) is warm when real inputs land.
  5. Optional K-probe matmuls at the very end (PROBES flag) to measure
     matmul cost vs K for the next iteration's design.
"""

import numpy as np

B, T, C = 64, 576, 1024
H = T // 2                   # 288, folded K
NCORES = 8
BPC = B // NCORES            # batches per core
Q = 0.8
PROBES = True

_CACHED = {}


def _dct_mat(N):
    n = np.arange(N)
    Mm = np.cos(np.pi * (2 * n[None, :] + 1) * n[:, None] / (2 * N))
    s = np.full(N, np.sqrt(2.0 / N))
    s[0] = np.sqrt(1.0 / N)
    return s[:, None] * Mm          # float64


def _build_weights(L):
    """Wu [H+ns1, 288] = [Me; pad; Pe'], Wv [H+ns2, 288] = [Mo; pad; Po'].
    The y block is zero-padded up to H=288 rows so the state block starts at
    a 32-aligned PSUM partition in every m-tile."""
    M64 = _dct_mat(T)
    Mi = _dct_mat(L)
    ke = np.arange(0, L, 2)
    ko = np.arange(1, L, 2)
    Pe = np.einsum('kj,kt->jt', Mi[ke, :], M64[ke, :])
    Po = np.einsum('kj,kt->jt', Mi[ko, :], M64[ko, :])
    ns1 = (L + 1) // 2
    ns2 = L // 2
    pe_u = np.zeros((H - len(ke), H))
    pe_v = np.zeros((H - len(ko), H))
    Wu = np.concatenate([M64[ke][:, :H], pe_u, Pe[:ns1, :H]], axis=0)
    Wv = np.concatenate([M64[ko][:, :H], pe_v, Po[:ns2, :H]], axis=0)
    return Wu, Wv


def _build_nc(L):
    """Bass program for truncation length L (574 for the seed-0 input).

    Inputs host-packed as in v2:
      xu/xv  [2, 2, 128, 4C] f16: (q, ki, p, (b c))
      xur/xvr [2, 128, C]: K-remainder rows of 4 batches packed on partitions
      wub/wvb [128, 2M]: cols (ki m); wur/wvr [128, M]: rem rows 4x-replic.
    Outputs (v3): yy/ss [2, L, 4, C] f16 -- quad-major so one dma_start per
    (q, t, m-tile, dest) ships 4 batches with 8KB-contiguous DRAM lines.
    """
    import concourse.bacc as bacc
    import concourse.mybir as mybir
    import concourse.tile as tile

    f16 = mybir.dt.float16
    f32 = mybir.dt.float32

    ns1 = (L + 1) // 2
    ns2 = L // 2
    MU = H + ns1
    MV = H + ns2
    MW = {"u": MU, "v": MV}
    YB = {"u": ns1, "v": ns2}         # y rows per transform
    NT = [(0, 512), (512, 512)]
    MM = max(MU, MV)
    MT = [(m0, min(128, MM - m0)) for m0 in range(0, MM, 128)]

    f8 = mybir.dt.float8e3

    nc = bacc.Bacc("TRN2", target_bir_lowering=False, debug=False,
                   num_devices=NCORES)
    xu = nc.dram_tensor("xu", [2, 2, 128, 4 * C], f16, kind="ExternalInput")
    xv = nc.dram_tensor("xv", [2, 2, 128, 4 * C], f8, kind="ExternalInput")
    xur = nc.dram_tensor("xur", [2, 128, C], f16, kind="ExternalInput")
    xvr = nc.dram_tensor("xvr", [2, 128, C], f8, kind="ExternalInput")
    wub = nc.dram_tensor("wub", [128, 2 * MU], f16, kind="ExternalInput")
    wvb = nc.dram_tensor("wvb", [128, 2 * MV], f16, kind="ExternalInput")
    wur = nc.dram_tensor("wur", [128, MU], f16, kind="ExternalInput")
    wvr = nc.dram_tensor("wvr", [128, MV], f16, kind="ExternalInput")
    # one output tensor: plane t=0 holds [y-u rows; pad; a rows], t=1 holds
    # [y-v rows; pad; b rows] -- one dma_start per (q, t, m-tile)
    os_ = nc.dram_tensor("os", [2, 2, 576, 4 * C], f16,
                         kind="ExternalOutput")
    XD = {"u": (xu, xur, wub, wur), "v": (xv, xvr, wvb, wvr)}

    with tile.TileContext(nc) as tc:
        with (
            tc.tile_pool(name="wpool", bufs=1) as wpool,
            tc.tile_pool(name="xpool", bufs=1) as xpool,
            tc.tile_pool(name="opool", bufs=6) as opool,
            tc.tile_pool(name="ps", bufs=8, space="PSUM") as ps,
        ):
            # --- warmup immediately: memset on vector (idle at start) ---
            wz = wpool.tile([128, 128], f16, tag="wz", name="wz")
            nc.vector.memset(wz[:], 0.0)
            pwarm = ps.tile([128, 512], f32, tag="pt", name="pt")
            for _ in range(18):
                nc.tensor.matmul(pwarm[:, 0:128], wz[:], wz[:],
                                 start=True, stop=True)

            # --- input kicks, first-use order, spread across engines ---
            xt, rt, wt, wr = {}, {}, {}, {}

            def load_w(t, eng):
                _, _, wd, wrd = XD[t]
                w_ = wpool.tile([128, 2 * MW[t]], f16, tag=f"w{t}",
                                name=f"w{t}")
                eng.dma_start(w_[:], wd[:, :])
                wt[t] = w_
                w_ = wpool.tile([128, MW[t]], f16, tag=f"w{t}r",
                                name=f"w{t}r")
                eng.dma_start(w_[:], wrd[:, :])
                wr[t] = w_

            load_w("v", nc.sync)     # v runs first
            # inputs mostly on the Act (scalar) HWDGE ring; the FIRST
            # wave's (q0, v) kicks go fine-grained across all 3 rings so
            # compute starts as early as possible
            xdt = {"u": f16, "v": f8}
            xd, rd, _, _ = XD["v"]
            for ki in range(2):
                x_ = xpool.tile([128, 4 * C], f8, tag=f"xv0{ki}",
                                name=f"xv0{ki}")
                xt[("v", 0, ki)] = x_
            r_ = xpool.tile([128, C], f8, tag="xvr0", name="xvr0")
            nc.gpsimd.dma_start(r_[:], rd[0, :, :])
            rt[("v", 0)] = r_
            rings = [nc.scalar, nc.sync, nc.gpsimd]
            j = 0
            for cb in range(4):
                for ki in range(2):
                    rings[j % 3].dma_start(
                        xt[("v", 0, ki)][:, cb * C:(cb + 1) * C],
                        xd[0, ki, :, cb * C:(cb + 1) * C])
                    j += 1
            load_w("u", nc.sync)
            for q in range(2):
                for t in ("v", "u"):
                    if q == 0 and t == "v":
                        continue
                    xd, rd, _, _ = XD[t]
                    for ki in range(2):
                        x_ = xpool.tile([128, 4 * C], xdt[t],
                                        tag=f"x{t}{q}{ki}",
                                        name=f"x{t}{q}{ki}")
                        nc.scalar.dma_start(x_[:, 0:2 * C],
                                            xd[q, ki, :, 0:2 * C])
                        xt[(t, q, ki)] = x_
                    r_ = xpool.tile([128, C], xdt[t], tag=f"x{t}r{q}",
                                    name=f"x{t}r{q}")
                    nc.gpsimd.dma_start(r_[:], rd[q, :, :])
                    rt[(t, q)] = r_
                    for ki in range(2):
                        nc.scalar.dma_start(xt[(t, q, ki)][:, 2 * C:4 * C],
                                            xd[q, ki, :, 2 * C:4 * C])

            def vcopy(dst, src):
                nc.vector.tensor_copy(dst, src)

            def scopy(dst, src):
                nc.scalar.copy(dst, src)

            oengs = [nc.sync, nc.gpsimd]
            ok_i = 0     # output call counter (engine rotation)
            pending = None   # delayed output call issued via scalar ring

            # --- compute waves: (q, t, m), 2 n-halves x 4 batches ---
            # v first: its fp8 kicks are half the bytes, so the first
            # wave's inputs land earliest
            NW = 20          # total waves
            for q in range(2):
                for t in ("v", "u"):
                    ti = 0 if t == "u" else 1
                    mw = MW[t]
                    for mi, (m0, mm) in enumerate(MT):
                        mmt = min(mm, mw - m0)
                        if mmt <= 0:
                            continue
                        stage = opool.tile([128, 4 * C], f16,
                                           tag="o", name="o")
                        for ni, (n0, nn) in enumerate(NT):
                            # ramp: first m-tile of the run goes in 2-bank
                            # halves so compute starts on half the inputs
                            groups = ([(0, 1), (2, 3)]
                                      if (q == 0 and t == "v" and mi == 0)
                                      else [(0, 1, 2, 3)])
                            pts = {}
                            for grp in groups:
                                for bi in grp:
                                    pts[bi] = ps.tile([128, 512], f32,
                                                      tag="pt", name="pt")
                                for ki in range(2):
                                    wsl = wt[t][:, ki * mw + m0:
                                                ki * mw + m0 + mmt]
                                    for bi in grp:
                                        nc.tensor.matmul(
                                            pts[bi][0:mmt, :],
                                            wsl,
                                            xt[(t, q, ki)][:, bi * C + n0:
                                                           bi * C + n0 + nn],
                                            start=(ki == 0), stop=False)
                                for bi in grp:
                                    nc.tensor.matmul(
                                        pts[bi][0:mmt, :],
                                        wr[t][32 * bi:32 * bi + 32,
                                              m0:m0 + mmt],
                                        rt[(t, q)][32 * bi:32 * bi + 32,
                                                   n0:n0 + nn],
                                        start=False, stop=True,
                                        tile_position=(32 * bi, 0))
                            # stage columns laid out (ni, bi, 512): each
                            # n-half is contiguous, so tail waves can ship
                            # a half as soon as its drains complete
                            for bi in range(4):
                                cp = vcopy if bi % 2 == 0 else scopy
                                c0 = ni * 2 * C + bi * 512
                                cp(stage[0:mmt, c0:c0 + nn],
                                   pts[bi][0:mmt, :])
                            if ok_i >= NW - 4:
                                d = os_[q, ti, m0:m0 + mmt,
                                        ni * 2 * C:(ni + 1) * 2 * C]
                                oengs[ni % 2].dma_start(
                                    d, stage[0:mmt,
                                             ni * 2 * C:(ni + 1) * 2 * C])
                        # ONE dma_start per wave ships y+state rows of all
                        # 4 batches.  Rotation sync/gpsimd immediate; every
                        # third call goes via the scalar (Act) ring delayed
                        # ONE wave so its issue never blocks scalar drains.
                        if pending is not None:
                            nc.scalar.dma_start(*pending)
                            pending = None
                        d = os_[q, ti, m0:m0 + mmt, :]
                        if ok_i >= NW - 4:
                            pass     # shipped per n-half above
                        elif ok_i % 3 == 2:
                            pending = (d, stage[0:mmt, :])
                        else:
                            oengs[ok_i % 3].dma_start(d, stage[0:mmt, :])
                        ok_i += 1
            if pending is not None:
                nc.scalar.dma_start(*pending)
                pending = None

    nc.finalize()
    return nc


def _get_nc(L):
    key = ("nc3", L)
    if key not in _CACHED:
        _CACHED[key] = _build_nc(L)
    return _CACHED[key]


def _ensure_trace_hook_safe():
    """If BASS_TRACE is set in the environment, run_bass_kernel_spmd imports
    antenv.axon_hooks, which may not exist. Install a working ctypes-based
    shim when possible, else disable tracing so the run cannot crash."""
    import os
    import sys
    import types

    if not os.environ.get("BASS_TRACE"):
        return
    try:
        import antenv.axon_hooks  # noqa: F401
        return
    except ImportError:
        pass
    try:
        from trn_agent_boot.trn_boot import _ntff_profile_via_ctypes
        hooks = types.ModuleType("antenv.axon_hooks")
        hook = _ntff_profile_via_ctypes("/opt/axon/libaxon_pjrt.so")
        hooks.get_axon_ntff_profile_hook = lambda: hook
        hooks.set_axon_ntff_profile_hook = lambda h: None
        sys.modules["antenv.axon_hooks"] = hooks
    except Exception:
        os.environ["BASS_NEVER_TRACE"] = "1"


def kernel(x: np.ndarray):
    from concourse.bass_utils import run_bass_kernel_spmd

    _ensure_trace_hook_safe()
    x = np.ascontiguousarray(np.asarray(x, dtype=np.float32))
    assert x.shape == (B, T, C)

    # ---- host: data-dependent truncation length L (tiny, exact math) ----
    M64 = _dct_mat(T)
    xbar = x.astype(np.float64).mean(axis=(0, 2))
    vq = np.abs(M64 @ xbar)
    thr = np.abs(np.quantile(vq, Q))
    idxs = np.where(vq > thr)[0]
    last_index = int(idxs[-1]) if idxs.size > 0 else -1
    L = last_index if last_index >= 0 else T - 1

    ns1 = (L + 1) // 2
    Wu, Wv = _build_weights(L)              # [H+ns1, 288], [H+ns2, 288]
    wu16 = np.ascontiguousarray(Wu.T).astype(np.float16)   # [288, H+ns1]
    wv16 = np.ascontiguousarray(Wv.T).astype(np.float16)

    # ---- host: fold input (u ships f16, v ships fp8 e3m4) ----
    import ml_dtypes
    xf = x[:, :H, :]
    xr = x[:, T - 1:H - 1:-1, :]
    u16 = (xf + xr).astype(np.float16)
    v16 = (xf - xr).astype(ml_dtypes.float8_e3m4)

    nc = _get_nc(L)

    def pack_x(z16):
        # [BPC,288,C] -> [2,2,128,4C] (q, ki, p, (b c)) + rem [2,128,C]
        full = z16[:, :256].reshape(2, 4, 2, 128, C)
        full = np.ascontiguousarray(full.transpose(0, 2, 3, 1, 4)
                                    ).reshape(2, 2, 128, 4 * C)
        remn = np.ascontiguousarray(z16[:, 256:288]).reshape(2, 128, C)
        return full, remn

    def pack_w(w16):
        # [288, M] -> [128, 2M] cols (ki m) + rem rows replicated [128, M]
        full = np.ascontiguousarray(w16[:256].reshape(2, 128, w16.shape[1])
                                    .transpose(1, 0, 2)
                                    ).reshape(128, 2 * w16.shape[1])
        remn = np.ascontiguousarray(np.tile(w16[256:288], (4, 1)))
        return full, remn

    wub_h, wur_h = pack_w(wu16)
    wvb_h, wvr_h = pack_w(wv16)
    in_maps = []
    for i in range(NCORES):
        xu_h, xur_h = pack_x(u16[i * BPC:(i + 1) * BPC])
        xv_h, xvr_h = pack_x(v16[i * BPC:(i + 1) * BPC])
        in_maps.append({"xu": xu_h, "xv": xv_h, "xur": xur_h, "xvr": xvr_h,
                        "wub": wub_h, "wvb": wvb_h,
                        "wur": wur_h, "wvr": wvr_h})
    res = run_bass_kernel_spmd(nc, in_maps, list(range(NCORES)))
    _CACHED["last_exec_time_ns"] = res.exec_time_ns

    # device layout os [2(q), 2(t), 576, 4, C]:
    #   t=0 rows [0:ns1]=y-even, [H:H+ns1]=a;  t=1 [0:ns2]=y-odd, [H:H+ns2]=b
    ns2 = L // 2

    def unq(o, tp, r0, rn):
        # stage cols (ni, bi, 512): [2, rn, 2, 4, 512] -> [BPC, rn, C]
        return o[:, tp, r0:r0 + rn, :].reshape(2, rn, 2, 4, 512) \
            .transpose(0, 3, 1, 2, 4).reshape(BPC, rn, C)

    osr = [np.asarray(res.results[i]["os"]).reshape(2, 2, 576, 4 * C)
           for i in range(NCORES)]
    ye = np.concatenate([unq(o, 0, 0, ns1) for o in osr], axis=0)
    yo = np.concatenate([unq(o, 1, 0, ns2) for o in osr], axis=0)
    aa = np.concatenate([unq(o, 0, H, ns1) for o in osr], axis=0)
    bb = np.concatenate([unq(o, 1, H, ns2) for o in osr], axis=0)

    x_dct_trunc = np.empty((B, L, C), dtype=np.float32)
    x_dct_trunc[:, 0::2, :] = ye.astype(np.float32)
    x_dct_trunc[:, 1::2, :] = yo.astype(np.float32)
    a32 = aa.astype(np.float32)
    b32 = bb.astype(np.float32)
    state = np.empty((B, L, C), dtype=np.float16)
    state[:, :ns2, :] = (a32[:, :ns2] + b32).astype(np.float16)
    if ns1 > ns2:
        state[:, ns2:ns1, :] = aa[:, ns2:ns1, :]   # lone middle row, L odd
    state[:, ns1:, :] = (a32[:, :ns2] - b32).astype(np.float16)[:, ::-1, :]
    return state, x_dct_trunc
